# revision 9
# baseline (speedup 1.0000x reference)
"""BitwiseResnet18 forward on 8 trn2 NeuronCores — pure batch data-parallel.

Numerics (matches jax-f32 CPU reference through every sign()):
- stem conv: exact-integer main term (a0*u0 on 8-bit grids in bf16 -> exact
  f32 PSUM integer accumulation) accumulated FIRST, then an algebraically
  exact f32 correction (a0*wr + xr*w) whose roundings are relative to the
  final value.  |err| <~ 1e-7, below the minimum sign margin (~1e-6).
- binarized convs: +-1 products in fp8, integer sums in f32 PSUM: exact.
- BN affines / residual adds: f32 mul-round-add-round on DVE, same as XLA.
"""
import copy
import numpy as np
import ml_dtypes

import concourse.bass as bass
import concourse.mybir as mybir
import concourse.tile as tile
from concourse.bass_utils import run_bass_kernel_spmd

NCORES = 8
B = 64          # per-core batch
CH = 8          # images per chunk (stages 2-4); stem/stage1 use CH//2
SC = CH // 2
F32 = mybir.dt.float32
BF16 = mybir.dt.bfloat16
FP8 = mybir.dt.float8e4
NP_FP8 = mybir.dt.np(FP8)
NP_BF16 = ml_dtypes.bfloat16

STAGES = [(64, 64, 1), (64, 128, 2), (128, 256, 2), (256, 512, 2)]


# --------------------------------------------------------------- wait splitter
def _split_waits(nc, max_waits=1):
    """The walrus build here accepts only one sync-wait per instruction;
    move extra waits onto injected EventSemaphore carriers."""
    mod = nc.m
    counter = [0]

    def carrier(engine, waits, debug):
        counter[0] += 1
        si = mybir.SyncInfo(on_wait=list(waits), on_update=[])
        return mybir.InstEventSemaphore(
            name=f"WSPLIT-{counter[0]}", engine=engine, sync_info=si,
            ins=[], outs=[], debug=debug)

    new_functions = []
    for function in mod.functions:
        nf = copy.replace(function, blocks=[])
        nf.set_allocations_from_list(function.allocations)
        for block in function.blocks:
            insts = []
            for inst in block.instructions:
                si = inst.sync_info
                waits = list(si.on_wait) if si is not None and si.on_wait else []
                if len(waits) > max_waits:
                    head, keep = waits[:-max_waits], waits[-max_waits:]
                    for i in range(0, len(head), max_waits):
                        insts.append(carrier(inst.engine, head[i:i + max_waits],
                                             inst.debug))
                    inst = copy.replace(inst, sync_info=mybir.SyncInfo(
                        on_wait=keep,
                        on_update=list(si.on_update) if si.on_update else []))
                insts.append(inst)
            nf.blocks.append(copy.replace(block, instructions=insts))
        new_functions.append(nf)
    new_mod = copy.replace(mod, functions=[])
    for f in new_functions:
        new_mod.functions.append(f)
    nc.m = new_mod
    return nc


# ------------------------------------------------------------------ host prep
def _f32(a):
    return np.asarray(a, dtype=np.float32)


def _bn_fold(p):
    var = _f32(p['var']); gamma = _f32(p['gamma'])
    beta = _f32(p['beta']); mean = _f32(p['mean'])
    inv = (np.float32(1.0) / np.sqrt(var + np.float32(1e-5))).astype(np.float32)
    s = (gamma * inv).astype(np.float32)
    t = (beta - (mean * gamma) * inv).astype(np.float32)
    return s, t


def _binw(w):
    w = np.asarray(w, dtype=np.float32)
    sw = np.where(w >= 0, np.float32(1), np.float32(-1))
    lhsT = np.transpose(sw, (1, 2, 3, 0)).reshape(sw.shape[1], 9, sw.shape[0])
    return np.ascontiguousarray(lhsT).astype(NP_FP8)          # [Cin, 9, Cout]


def _binw1x1(w):
    w = np.asarray(w, dtype=np.float32)[:, :, 0, 0]
    sw = np.where(w >= 0, np.float32(1), np.float32(-1))
    return np.ascontiguousarray(sw.T).astype(NP_FP8)          # [Cin, Cout]


def _prep_inputs(x, params):
    x = np.asarray(x, dtype=np.float32)
    x64 = x.astype(np.float64)
    a0 = np.round(x64 * 32.0) / 32.0          # 2^-5 grid, ints <= 173 (bf16 ok)
    xr = (x64 - a0).astype(np.float32)

    w64 = np.asarray(params['conv1_w'], np.float32).astype(np.float64)
    u0 = np.round(w64 * 256.0) / 256.0        # 2^-8 grid, ints <= 167 (bf16 ok)
    wr = (w64 - u0).astype(np.float32)
    wf = np.asarray(params['conv1_w'], np.float32)
    um = np.zeros((21, 7, 64), np.float64)    # [row, kw, Cout]
    wc = np.zeros((42, 7, 64), np.float32)
    for kw in range(7):
        for kh in range(7):
            for ci in range(3):
                r = kh * 3 + ci
                um[r, kw, :] = u0[:, ci, kh, kw]
                wc[r, kw, :] = wr[:, ci, kh, kw]
                wc[21 + r, kw, :] = wf[:, ci, kh, kw]
    rep = {'stwm': um.astype(NP_BF16), 'stwc': wc}

    s, t = _bn_fold(params['bn1'])
    rep['bn1s'] = s.reshape(64, 1)
    rep['bn1t'] = t.reshape(64, 1)

    st = params['stages']
    w1 = np.stack([_binw(st[0][0]['conv1_w']), _binw(st[0][0]['conv2_w']),
                   _binw(st[0][1]['conv1_w']), _binw(st[0][1]['conv2_w'])], axis=2)
    rep['s1w'] = np.ascontiguousarray(w1)                     # [64, 9, 4, 64]
    bs = [_bn_fold(st[0][0]['bn1']), _bn_fold(st[0][0]['bn2']),
          _bn_fold(st[0][1]['bn1']), _bn_fold(st[0][1]['bn2'])]
    rep['s1bs'] = np.ascontiguousarray(np.stack([b[0] for b in bs], 1))
    rep['s1bt'] = np.ascontiguousarray(np.stack([b[1] for b in bs], 1))

    for si, (cin, cout, stride) in enumerate(STAGES[1:], start=2):
        b1, b2 = st[si - 1]
        kin = (cin + 127) // 128
        kout = (cout + 127) // 128
        c1 = _binw(b1['conv1_w'])
        dw = _binw1x1(b1['down_w'])
        rest = np.stack([_binw(b1['conv2_w']), _binw(b2['conv1_w']),
                         _binw(b2['conv2_w'])], axis=2)       # [cout,9,3,cout]
        kp = min(cin, 128)
        rep[f's{si}c1w'] = np.ascontiguousarray(
            c1.reshape(kin, kp, 9, cout).transpose(1, 0, 2, 3))
        rep[f's{si}dw'] = np.ascontiguousarray(
            dw.reshape(kin, kp, cout).transpose(1, 0, 2))
        rep[f's{si}w'] = np.ascontiguousarray(
            rest.reshape(kout, 128, 9, 3, cout).transpose(1, 0, 2, 3, 4))
        bs = [_bn_fold(b1['bn1']), _bn_fold(b1['down_bn']), _bn_fold(b1['bn2']),
              _bn_fold(b2['bn1']), _bn_fold(b2['bn2'])]
        rep[f's{si}bs'] = np.ascontiguousarray(
            np.stack([b[0] for b in bs], 1).reshape(kout, 128, 5).transpose(1, 0, 2))
        rep[f's{si}bt'] = np.ascontiguousarray(
            np.stack([b[1] for b in bs], 1).reshape(kout, 128, 5).transpose(1, 0, 2))

    fcw = _f32(params['fc_w'])
    rep['fcw'] = np.ascontiguousarray(fcw.T.reshape(4, 128, 10).transpose(1, 0, 2))
    rep['fcb'] = _f32(params['fc_b']).reshape(10, 1)
    rep['fcs'] = _f32(params['scale']).reshape(10, 1)

    shards = []
    a0h = a0.astype(NP_BF16)
    a0f = a0.astype(np.float32)
    for c in range(NCORES):
        sl = slice(c * B, (c + 1) * B)
        shards.append({
            'xa0h': np.ascontiguousarray(a0h[sl].transpose(1, 0, 2, 3)),
            'xa0f': np.ascontiguousarray(a0f[sl].transpose(1, 0, 2, 3)),
            'xrf': np.ascontiguousarray(xr[sl].transpose(1, 0, 2, 3)),
        })
    return rep, shards


# ------------------------------------------------------------------ bass build
def _build():
    nc = bass.Bass()
    NCH = B // CH
    AT = mybir.ActivationFunctionType
    OP = mybir.AluOpType

    d = {}
    d['xa0h'] = nc.dram_tensor("xa0h", [3, B, 32, 32], BF16, kind="ExternalInput")
    d['xa0f'] = nc.dram_tensor("xa0f", [3, B, 32, 32], F32, kind="ExternalInput")
    d['xrf'] = nc.dram_tensor("xrf", [3, B, 32, 32], F32, kind="ExternalInput")
    d['stwm'] = nc.dram_tensor("stwm", [21, 7, 64], BF16, kind="ExternalInput")
    d['stwc'] = nc.dram_tensor("stwc", [42, 7, 64], F32, kind="ExternalInput")
    d['bn1s'] = nc.dram_tensor("bn1s", [64, 1], F32, kind="ExternalInput")
    d['bn1t'] = nc.dram_tensor("bn1t", [64, 1], F32, kind="ExternalInput")
    d['s1w'] = nc.dram_tensor("s1w", [64, 9, 4, 64], FP8, kind="ExternalInput")
    d['s1bs'] = nc.dram_tensor("s1bs", [64, 4], F32, kind="ExternalInput")
    d['s1bt'] = nc.dram_tensor("s1bt", [64, 4], F32, kind="ExternalInput")
    for si, (cin, cout, stride) in enumerate(STAGES[1:], start=2):
        kin = (cin + 127) // 128
        kout = (cout + 127) // 128
        kp = min(cin, 128)
        d[f's{si}c1w'] = nc.dram_tensor(f"s{si}c1w", [kp, kin, 9, cout], FP8, kind="ExternalInput")
        d[f's{si}dw'] = nc.dram_tensor(f"s{si}dw", [kp, kin, cout], FP8, kind="ExternalInput")
        d[f's{si}w'] = nc.dram_tensor(f"s{si}w", [128, kout, 9, 3, cout], FP8, kind="ExternalInput")
        d[f's{si}bs'] = nc.dram_tensor(f"s{si}bs", [128, kout, 5], F32, kind="ExternalInput")
        d[f's{si}bt'] = nc.dram_tensor(f"s{si}bt", [128, kout, 5], F32, kind="ExternalInput")
    d['fcw'] = nc.dram_tensor("fcw", [128, 4, 10], F32, kind="ExternalInput")
    d['fcb'] = nc.dram_tensor("fcb", [10, 1], F32, kind="ExternalInput")
    d['fcs'] = nc.dram_tensor("fcs", [10, 1], F32, kind="ExternalInput")
    out_t = nc.dram_tensor("out", [10, B], F32, kind="ExternalOutput")

    with tile.TileContext(nc) as tc:
        import contextlib
        ctx = contextlib.ExitStack()
        wpool = ctx.enter_context(tc.tile_pool(name="w", bufs=1))
        apool = ctx.enter_context(tc.tile_pool(name="a", bufs=1))
        tpool = ctx.enter_context(tc.tile_pool(name="t", bufs=2))
        ppool = ctx.enter_context(tc.tile_pool(name="p", bufs=6, space="PSUM"))
        pfc = ctx.enter_context(tc.tile_pool(name="pfc", bufs=1, space="PSUM"))

        W = {}
        for k in d:
            if k in ('xa0h', 'xa0f', 'xrf'):
                continue
            W[k] = wpool.tile(list(d[k].shape), d[k].dtype, tag=k, name=k)
            nc.sync.dma_start(W[k][:], d[k][:])

        Rm = apool.tile([21, SC, 32, 38], BF16, tag="Rm")
        Rc = apool.tile([42, SC, 32, 38], F32, tag="Rc")
        res1 = apool.tile([64, SC, 32, 32], F32, tag="res1")
        bin1 = apool.tile([64, CH, 34, 34], FP8, tag="bin1")
        bin1t = apool.tile([64, CH, 34, 34], FP8, tag="bin1t")
        res2 = apool.tile([128, 1, CH, 16, 16], F32, tag="res2")
        bin2 = apool.tile([128, 1, CH, 18, 18], FP8, tag="bin2")
        bin2t = apool.tile([128, 1, CH, 18, 18], FP8, tag="bin2t")
        res3 = apool.tile([128, 2, CH, 8, 8], F32, tag="res3")
        bin3 = apool.tile([128, 2, CH, 10, 10], FP8, tag="bin3")
        bin3t = apool.tile([128, 2, CH, 10, 10], FP8, tag="bin3t")
        res4 = apool.tile([128, 4, CH, 4, 4], F32, tag="res4")
        bin4 = apool.tile([128, 4, CH, 6, 6], FP8, tag="bin4")
        bin4t = apool.tile([128, 4, CH, 6, 6], FP8, tag="bin4t")
        hsg = apool.tile([128, 4, CH, 16], BF16, tag="hsg")
        pooled = apool.tile([128, 4, B], F32, tag="pooled")

        for buf in (Rm, Rc, bin1, bin1t, bin2, bin2t, bin3, bin3t, bin4, bin4t):
            nc.vector.memset(buf[:], 0.0)

        def sgn(out_ap, in_ap, s=1.0, b=0.0):
            nc.scalar.activation(out_ap, in_ap, AT.Sign, bias=b, scale=s)

        def psum(npart, nfree):
            ps = ppool.tile([128, 512], F32, tag="ps")
            return ps[:npart, :nfree]

        for c in range(NCH):
            # ======== STEM + STAGE1 in half-chunks of SC images ========
            for sc in range(2):
                goff = c * CH + sc * SC          # global image offset in B
                loff = sc * SC                   # offset within chunk buffers
                gsl = slice(goff, goff + SC)
                for kh in range(7):
                    y0 = max(0, 3 - kh); y1 = min(31, 34 - kh)
                    sy0 = y0 + kh - 3
                    n_y = y1 - y0 + 1
                    for im in range(SC):
                        gim = goff + im
                        nc.sync.dma_start(
                            Rm[3 * kh:3 * kh + 3, im, y0:y1 + 1, 3:35],
                            d['xa0h'][:, gim, sy0:sy0 + n_y, :])
                        nc.sync.dma_start(
                            Rc[3 * kh:3 * kh + 3, im, y0:y1 + 1, 3:35],
                            d['xa0f'][:, gim, sy0:sy0 + n_y, :])
                        nc.sync.dma_start(
                            Rc[21 + 3 * kh:21 + 3 * kh + 3, im, y0:y1 + 1, 3:35],
                            d['xrf'][:, gim, sy0:sy0 + n_y, :])
                for img in range(SC):
                    bimg = loff + img            # index into bin1 (chunk-local)
                    for yh in range(2):
                        ys = 16 * yh
                        pv = psum(64, 512).rearrange("p (y x) -> p y x", y=16)
                        for kw in range(7):
                            nc.tensor.matmul(
                                pv[:], W['stwm'][:, kw, :],
                                Rm[:, img, ys:ys + 16, kw:kw + 32],
                                start=(kw == 0), stop=False)
                        for kw in range(7):
                            nc.tensor.matmul(
                                pv[:], W['stwc'][:, kw, :],
                                Rc[:, img, ys:ys + 16, kw:kw + 32],
                                start=False, stop=(kw == 6))
                        nc.vector.tensor_scalar(
                            res1[:, img, ys:ys + 16, :], pv[:],
                            W['bn1s'][:, 0:1], W['bn1t'][:, 0:1], OP.mult, OP.add)
                        sgn(bin1[:, bimg, 1 + ys:17 + ys, 1:33],
                            res1[:, img, ys:ys + 16, :])

                # -------- stage 1 (64ch 32x32), two blocks --------
                def conv3_64(pv, src, bimg, yh, widx):
                    i = 0
                    for kh in range(3):
                        for kw in range(3):
                            nc.tensor.matmul(
                                pv[:], W['s1w'][:, kh * 3 + kw, widx, :],
                                src[:, bimg, kh + 16 * yh:kh + 16 * yh + 16,
                                    kw:kw + 32],
                                start=(i == 0), stop=(i == 8))
                            i += 1
                for blk in range(2):
                    c1, c2 = 2 * blk, 2 * blk + 1
                    for img in range(SC):
                        bimg = loff + img
                        for yh in range(2):
                            pv = psum(64, 512).rearrange("p (y x) -> p y x", y=16)
                            conv3_64(pv, bin1, bimg, yh, c1)
                            sgn(bin1t[:, bimg, 1 + 16 * yh:17 + 16 * yh, 1:33],
                                pv[:], W['s1bs'][:, c1:c1 + 1],
                                W['s1bt'][:, c1:c1 + 1])
                    for img in range(SC):
                        bimg = loff + img
                        for yh in range(2):
                            ys = 16 * yh
                            pv = psum(64, 512).rearrange("p (y x) -> p y x", y=16)
                            conv3_64(pv, bin1t, bimg, yh, c2)
                            h2 = tpool.tile([64, 16, 32], F32, tag="h2a")
                            nc.vector.tensor_scalar(
                                h2[:], pv[:], W['s1bs'][:, c2:c2 + 1],
                                W['s1bt'][:, c2:c2 + 1], OP.mult, OP.add)
                            rsl = res1[:, img, ys:ys + 16, :]
                            nc.vector.tensor_tensor(rsl, h2[:], rsl, OP.add)
                            sgn(bin1[:, bimg, 1 + ys:17 + ys, 1:33], rsl)

            # ======== STAGES 2..4 on the full chunk ========
            def stage(si, cin, cout, get_prev, resp, binc, binct, H):
                kin = (cin + 127) // 128
                kout = (cout + 127) // 128
                px = CH * H * H
                ntile = max(1, px // 512)
                npt = CH // ntile
                Ws, Wt = W[f's{si}bs'], W[f's{si}bt']
                nfree = npt * H * H

                def pview(npart=128):
                    return psum(npart, nfree).rearrange(
                        "p (n y x) -> p n y x", n=npt, y=H)

                def srb(buf, g, ns2, kh, kw):   # stride-1 rhs from own stage buf
                    return buf[:, g, ns2, kh:kh + H, kw:kw + H]

                for tidx in range(ntile):
                    ns2 = slice(tidx * npt, (tidx + 1) * npt)
                    for m in range(kout):
                        mp = slice(128 * m, 128 * m + 128)
                        pv = pview()
                        i = 0
                        for g in range(kin):
                            for kh in range(3):
                                for kw in range(3):
                                    nc.tensor.matmul(
                                        pv[:], W[f's{si}c1w'][:, g, kh * 3 + kw, mp],
                                        get_prev(g, ns2, kh, kw, 2),
                                        start=(i == 0), stop=(i == kin * 9 - 1))
                                    i += 1
                        sgn(binct[:, m, ns2, 1:1 + H, 1:1 + H], pv[:],
                            Ws[:, m, 0:1], Wt[:, m, 0:1])
                        pvd = pview()
                        for g in range(kin):
                            nc.tensor.matmul(
                                pvd[:], W[f's{si}dw'][:, g, mp],
                                get_prev(g, ns2, 1, 1, 2),
                                start=(g == 0), stop=(g == kin - 1))
                        nc.vector.tensor_scalar(
                            resp[:, m, ns2, :, :], pvd[:],
                            Ws[:, m, 1:2], Wt[:, m, 1:2], OP.mult, OP.add)

                for cidx in range(3):
                    srcbuf = binct if cidx in (0, 2) else binc
                    dstbuf = binc if cidx in (0, 2) else binct
                    bnc = 2 + cidx
                    for tidx in range(ntile):
                        ns2 = slice(tidx * npt, (tidx + 1) * npt)
                        for m in range(kout):
                            mp = slice(128 * m, 128 * m + 128)
                            pv = pview()
                            i = 0
                            for g in range(kout):
                                for kh in range(3):
                                    for kw in range(3):
                                        nc.tensor.matmul(
                                            pv[:],
                                            W[f's{si}w'][:, g, kh * 3 + kw, cidx, mp],
                                            srb(srcbuf, g, ns2, kh, kw),
                                            start=(i == 0),
                                            stop=(i == kout * 9 - 1))
                                        i += 1
                            if cidx in (0, 2):
                                h2 = tpool.tile([128, 512], F32, tag="h2b")
                                h2v = h2[:, :nfree].rearrange(
                                    "p (n y x) -> p n y x", n=npt, y=H)
                                nc.vector.tensor_scalar(
                                    h2v[:], pv[:], Ws[:, m, bnc:bnc + 1],
                                    Wt[:, m, bnc:bnc + 1], OP.mult, OP.add)
                                rsl = resp[:, m, ns2, :, :]
                                nc.vector.tensor_tensor(rsl, h2v[:], rsl, OP.add)
                                sgn(dstbuf[:, m, ns2, 1:1 + H, 1:1 + H], rsl)
                            else:
                                sgn(dstbuf[:, m, ns2, 1:1 + H, 1:1 + H], pv[:],
                                    Ws[:, m, bnc:bnc + 1], Wt[:, m, bnc:bnc + 1])

            def prev1(g, ns2, kh, kw, stride):   # from bin1 (4D, chunk-local)
                return bin1[:, ns2, kh:kh + 31:2, kw:kw + 31:2]

            def prev2(g, ns2, kh, kw, stride):
                return bin2[:, 0, ns2, kh:kh + 15:2, kw:kw + 15:2]

            def prev3(g, ns2, kh, kw, stride):
                return bin3[:, g, ns2, kh:kh + 7:2, kw:kw + 7:2]

            stage(2, 64, 128, prev1, res2, bin2, bin2t, 16)
            stage(3, 128, 256, prev2, res3, bin3, bin3t, 8)
            stage(4, 256, 512, prev3, res4, bin4, bin4t, 4)

            # ======== POOL ========
            for m in range(4):
                sgn(hsg[:, m, :, :],
                    res4[:, m, :, :, :].rearrange("p n y x -> p n (y x)"))
            acc = tpool.tile([128, 4, CH], F32, tag="poolacc")
            nc.vector.tensor_copy(acc[:], hsg[:, :, :, 0])
            for i in range(1, 16):
                nc.vector.tensor_tensor(acc[:], acc[:], hsg[:, :, :, i], OP.add)
            nc.scalar.mul(pooled[:, :, c * CH:(c + 1) * CH], acc[:], 1.0 / 16.0)

        # ======== FC ========
        psf = pfc.tile([10, B], F32, tag="ps_fc")
        for g in range(4):
            nc.tensor.matmul(psf[:], W['fcw'][:, g, :], pooled[:, g, :],
                             start=(g == 0), stop=(g == 3))
        osb = tpool.tile([10, B], F32, tag="osb")
        nc.vector.tensor_scalar(osb[:], psf[:], W['fcb'][:, 0:1],
                                W['fcs'][:, 0:1], OP.add, OP.mult)
        nc.sync.dma_start(out_t[:], osb[:])
        ctx.close()

    _split_waits(nc)
    return nc


_NC_CACHE = None


def _get_nc():
    global _NC_CACHE
    if _NC_CACHE is None:
        _NC_CACHE = _build()
    return _NC_CACHE


def kernel(x, params, _trace=False):
    rep, shards = _prep_inputs(x, params)
    nc = _get_nc()
    in_maps = []
    for c in range(NCORES):
        m = dict(rep)
        m.update(shards[c])
        in_maps.append(m)
    res = run_bass_kernel_spmd(nc, in_maps, core_ids=list(range(NCORES)))
    outs = [r['out'].T for r in res.results]
    full = np.ascontiguousarray(np.concatenate(outs, axis=0), dtype=np.float32)
    if _trace:
        return full, res
    return full


# revision 10
# speedup vs baseline: 1.1087x; 1.1087x over previous
"""BitwiseResnet18 forward on 8 trn2 NeuronCores — pure batch data-parallel.

Numerics (matches jax-f32 CPU reference through every sign()):
- stem conv: exact-integer main term (a0*u0 on 8-bit grids in bf16 -> exact
  f32 PSUM integer accumulation) accumulated FIRST, then an algebraically
  exact f32 correction (a0*wr + xr*w) whose roundings are relative to the
  final value.  |err| <~ 1e-7, below the minimum sign margin (~1e-6).
- binarized convs: +-1 products in fp8, integer sums in f32 PSUM: exact.
- BN affines / residual adds: f32 mul-round-add-round on DVE, same as XLA.
"""
import copy
import numpy as np
import ml_dtypes

import concourse.bass as bass
import concourse.mybir as mybir
import concourse.tile as tile
from concourse.bass_utils import run_bass_kernel_spmd

NCORES = 8
B = 64          # per-core batch
CH = 8          # images per chunk (stages 2-4); stem/stage1 use CH//2
SC = CH // 2
F32 = mybir.dt.float32
BF16 = mybir.dt.bfloat16
FP8 = mybir.dt.float8e4
NP_FP8 = mybir.dt.np(FP8)
NP_BF16 = ml_dtypes.bfloat16

STAGES = [(64, 64, 1), (64, 128, 2), (128, 256, 2), (256, 512, 2)]


# --------------------------------------------------------------- wait splitter
def _split_waits(nc, max_waits=1):
    """The walrus build here accepts only one sync-wait per instruction;
    move extra waits onto injected EventSemaphore carriers."""
    mod = nc.m
    counter = [0]

    def carrier(engine, waits, debug):
        counter[0] += 1
        si = mybir.SyncInfo(on_wait=list(waits), on_update=[])
        return mybir.InstEventSemaphore(
            name=f"WSPLIT-{counter[0]}", engine=engine, sync_info=si,
            ins=[], outs=[], debug=debug)

    new_functions = []
    for function in mod.functions:
        nf = copy.replace(function, blocks=[])
        nf.set_allocations_from_list(function.allocations)
        for block in function.blocks:
            insts = []
            for inst in block.instructions:
                si = inst.sync_info
                waits = list(si.on_wait) if si is not None and si.on_wait else []
                if len(waits) > max_waits:
                    head, keep = waits[:-max_waits], waits[-max_waits:]
                    for i in range(0, len(head), max_waits):
                        insts.append(carrier(inst.engine, head[i:i + max_waits],
                                             inst.debug))
                    inst = copy.replace(inst, sync_info=mybir.SyncInfo(
                        on_wait=keep,
                        on_update=list(si.on_update) if si.on_update else []))
                insts.append(inst)
            nf.blocks.append(copy.replace(block, instructions=insts))
        new_functions.append(nf)
    new_mod = copy.replace(mod, functions=[])
    for f in new_functions:
        new_mod.functions.append(f)
    nc.m = new_mod
    return nc


# ------------------------------------------------------------------ host prep
def _f32(a):
    return np.asarray(a, dtype=np.float32)


def _bn_fold(p):
    var = _f32(p['var']); gamma = _f32(p['gamma'])
    beta = _f32(p['beta']); mean = _f32(p['mean'])
    inv = (np.float32(1.0) / np.sqrt(var + np.float32(1e-5))).astype(np.float32)
    s = (gamma * inv).astype(np.float32)
    t = (beta - (mean * gamma) * inv).astype(np.float32)
    return s, t


def _binw(w):
    w = np.asarray(w, dtype=np.float32)
    sw = np.where(w >= 0, np.float32(1), np.float32(-1))
    lhsT = np.transpose(sw, (1, 2, 3, 0)).reshape(sw.shape[1], 9, sw.shape[0])
    return np.ascontiguousarray(lhsT).astype(NP_FP8)          # [Cin, 9, Cout]


def _binw1x1(w):
    w = np.asarray(w, dtype=np.float32)[:, :, 0, 0]
    sw = np.where(w >= 0, np.float32(1), np.float32(-1))
    return np.ascontiguousarray(sw.T).astype(NP_FP8)          # [Cin, Cout]


def _prep_inputs(x, params):
    x = np.asarray(x, dtype=np.float32)
    x64 = x.astype(np.float64)
    a0 = np.round(x64 * 32.0) / 32.0          # 2^-5 grid, ints <= 173
    resid = x64 - a0
    xrh_s = (resid * 2.0 ** 12).astype(np.float16)       # scaled limb
    xrh = xrh_s.astype(np.float64) * 2.0 ** -12
    xrl_s = ((resid - xrh) * 2.0 ** 24).astype(np.float16)

    w64 = np.asarray(params['conv1_w'], np.float32).astype(np.float64)
    u0 = np.round(w64 * 256.0) / 256.0        # 2^-8 grid, ints <= 167 (bf16 ok)
    wr = w64 - u0
    wr1 = np.round(wr * 2.0 ** 16) * 2.0 ** -16
    wr2 = np.round((wr - wr1) * 2.0 ** 24) * 2.0 ** -24
    wr3 = np.round((wr - wr1 - wr2) * 2.0 ** 32) * 2.0 ** -32
    um = np.zeros((21, 7, 64), np.float64)    # [row, kw, Cout]
    wa = np.zeros((126, 7, 64), np.float64)
    wb = np.zeros((42, 7, 64), np.float64)
    for kw in range(7):
        for kh in range(7):
            for ci in range(3):
                r = kh * 3 + ci
                um[r, kw, :] = u0[:, ci, kh, kw]
                wa[r, kw, :] = u0[:, ci, kh, kw] * 2.0 ** -12      # x xrh_s
                wa[21 + r, kw, :] = u0[:, ci, kh, kw] * 2.0 ** -24  # x xrl_s
                wa[42 + r, kw, :] = wr1[:, ci, kh, kw]              # x a0
                wa[63 + r, kw, :] = wr2[:, ci, kh, kw] * 2.0 ** 8   # x a0*2^-8
                wa[84 + r, kw, :] = wr3[:, ci, kh, kw] * 2.0 ** 16  # x a0*2^-16
                wa[105 + r, kw, :] = wr1[:, ci, kh, kw] * 2.0 ** -12  # x xrh_s
                wb[r, kw, :] = wr2[:, ci, kh, kw] * 2.0 ** -12      # x xrh_s
                wb[21 + r, kw, :] = wr1[:, ci, kh, kw] * 2.0 ** -24  # x xrl_s
    rep = {'stwm': um.astype(NP_BF16), 'stwa': wa.astype(NP_BF16),
           'stwb': wb.astype(NP_BF16)}

    s, t = _bn_fold(params['bn1'])
    rep['bn1s'] = s.reshape(64, 1)
    rep['bn1t'] = t.reshape(64, 1)

    st = params['stages']
    w1 = np.stack([_binw(st[0][0]['conv1_w']), _binw(st[0][0]['conv2_w']),
                   _binw(st[0][1]['conv1_w']), _binw(st[0][1]['conv2_w'])], axis=2)
    rep['s1w'] = np.ascontiguousarray(w1)                     # [64, 9, 4, 64]
    bs = [_bn_fold(st[0][0]['bn1']), _bn_fold(st[0][0]['bn2']),
          _bn_fold(st[0][1]['bn1']), _bn_fold(st[0][1]['bn2'])]
    rep['s1bs'] = np.ascontiguousarray(np.stack([b[0] for b in bs], 1))
    rep['s1bt'] = np.ascontiguousarray(np.stack([b[1] for b in bs], 1))

    for si, (cin, cout, stride) in enumerate(STAGES[1:], start=2):
        b1, b2 = st[si - 1]
        kin = (cin + 127) // 128
        kout = (cout + 127) // 128
        c1 = _binw(b1['conv1_w'])
        dw = _binw1x1(b1['down_w'])
        rest = np.stack([_binw(b1['conv2_w']), _binw(b2['conv1_w']),
                         _binw(b2['conv2_w'])], axis=2)       # [cout,9,3,cout]
        kp = min(cin, 128)
        rep[f's{si}c1w'] = np.ascontiguousarray(
            c1.reshape(kin, kp, 9, cout).transpose(1, 0, 2, 3))
        rep[f's{si}dw'] = np.ascontiguousarray(
            dw.reshape(kin, kp, cout).transpose(1, 0, 2))
        rep[f's{si}w'] = np.ascontiguousarray(
            rest.reshape(kout, 128, 9, 3, cout).transpose(1, 0, 2, 3, 4))
        bs = [_bn_fold(b1['bn1']), _bn_fold(b1['down_bn']), _bn_fold(b1['bn2']),
              _bn_fold(b2['bn1']), _bn_fold(b2['bn2'])]
        rep[f's{si}bs'] = np.ascontiguousarray(
            np.stack([b[0] for b in bs], 1).reshape(kout, 128, 5).transpose(1, 0, 2))
        rep[f's{si}bt'] = np.ascontiguousarray(
            np.stack([b[1] for b in bs], 1).reshape(kout, 128, 5).transpose(1, 0, 2))

    fcw = _f32(params['fc_w'])
    rep['fcw'] = np.ascontiguousarray(fcw.T.reshape(4, 128, 10).transpose(1, 0, 2))
    rep['fcb'] = _f32(params['fc_b']).reshape(10, 1)
    rep['fcs'] = _f32(params['scale']).reshape(10, 1)

    shards = []
    a0h = a0.astype(np.float16)
    a08 = (a0 * 2.0 ** -8).astype(np.float16)
    a016 = (a0 * 2.0 ** -16).astype(np.float16)
    for c in range(NCORES):
        sl = slice(c * B, (c + 1) * B)
        shards.append({
            'xa0': np.ascontiguousarray(a0h[sl].transpose(1, 0, 2, 3)),
            'xa08': np.ascontiguousarray(a08[sl].transpose(1, 0, 2, 3)),
            'xa016': np.ascontiguousarray(a016[sl].transpose(1, 0, 2, 3)),
            'xrh': np.ascontiguousarray(xrh_s[sl].transpose(1, 0, 2, 3)),
            'xrl': np.ascontiguousarray(xrl_s[sl].transpose(1, 0, 2, 3)),
        })
    return rep, shards


# ------------------------------------------------------------------ bass build
def _build():
    nc = bass.Bass()
    NCH = B // CH
    AT = mybir.ActivationFunctionType
    OP = mybir.AluOpType

    d = {}
    FP16 = mybir.dt.float16
    for nm in ('xa0', 'xa08', 'xa016', 'xrh', 'xrl'):
        d[nm] = nc.dram_tensor(nm, [3, B, 32, 32], FP16, kind="ExternalInput")
    d['stwm'] = nc.dram_tensor("stwm", [21, 7, 64], BF16, kind="ExternalInput")
    d['stwa'] = nc.dram_tensor("stwa", [126, 7, 64], BF16, kind="ExternalInput")
    d['stwb'] = nc.dram_tensor("stwb", [42, 7, 64], BF16, kind="ExternalInput")
    d['bn1s'] = nc.dram_tensor("bn1s", [64, 1], F32, kind="ExternalInput")
    d['bn1t'] = nc.dram_tensor("bn1t", [64, 1], F32, kind="ExternalInput")
    d['s1w'] = nc.dram_tensor("s1w", [64, 9, 4, 64], FP8, kind="ExternalInput")
    d['s1bs'] = nc.dram_tensor("s1bs", [64, 4], F32, kind="ExternalInput")
    d['s1bt'] = nc.dram_tensor("s1bt", [64, 4], F32, kind="ExternalInput")
    for si, (cin, cout, stride) in enumerate(STAGES[1:], start=2):
        kin = (cin + 127) // 128
        kout = (cout + 127) // 128
        kp = min(cin, 128)
        d[f's{si}c1w'] = nc.dram_tensor(f"s{si}c1w", [kp, kin, 9, cout], FP8, kind="ExternalInput")
        d[f's{si}dw'] = nc.dram_tensor(f"s{si}dw", [kp, kin, cout], FP8, kind="ExternalInput")
        d[f's{si}w'] = nc.dram_tensor(f"s{si}w", [128, kout, 9, 3, cout], FP8, kind="ExternalInput")
        d[f's{si}bs'] = nc.dram_tensor(f"s{si}bs", [128, kout, 5], F32, kind="ExternalInput")
        d[f's{si}bt'] = nc.dram_tensor(f"s{si}bt", [128, kout, 5], F32, kind="ExternalInput")
    d['fcw'] = nc.dram_tensor("fcw", [128, 4, 10], F32, kind="ExternalInput")
    d['fcb'] = nc.dram_tensor("fcb", [10, 1], F32, kind="ExternalInput")
    d['fcs'] = nc.dram_tensor("fcs", [10, 1], F32, kind="ExternalInput")
    out_t = nc.dram_tensor("out", [10, B], F32, kind="ExternalOutput")

    with tile.TileContext(nc) as tc:
        import contextlib
        ctx = contextlib.ExitStack()
        wpool = ctx.enter_context(tc.tile_pool(name="w", bufs=1))
        apool = ctx.enter_context(tc.tile_pool(name="a", bufs=1))
        tpool = ctx.enter_context(tc.tile_pool(name="t", bufs=2))
        ppool = ctx.enter_context(tc.tile_pool(name="p", bufs=6, space="PSUM"))
        pfc = ctx.enter_context(tc.tile_pool(name="pfc", bufs=1, space="PSUM"))

        W = {}
        for k in d:
            if k in ('xa0', 'xa08', 'xa016', 'xrh', 'xrl'):
                continue
            W[k] = wpool.tile(list(d[k].shape), d[k].dtype, tag=k, name=k)
            nc.sync.dma_start(W[k][:], d[k][:])

        Rm = apool.tile([21, SC, 32, 38], FP16, tag="Rm")
        Rk = apool.tile([126, SC, 32, 38], FP16, tag="Rk")
        res1 = apool.tile([64, SC, 32, 32], F32, tag="res1")
        bin1 = apool.tile([64, CH, 34, 34], FP8, tag="bin1")
        bin1t = apool.tile([64, CH, 34, 34], FP8, tag="bin1t")
        res2 = apool.tile([128, 1, CH, 16, 16], F32, tag="res2")
        bin2 = apool.tile([128, 1, CH, 18, 18], FP8, tag="bin2")
        bin2t = apool.tile([128, 1, CH, 18, 18], FP8, tag="bin2t")
        res3 = apool.tile([128, 2, CH, 8, 8], F32, tag="res3")
        bin3 = apool.tile([128, 2, CH, 10, 10], FP8, tag="bin3")
        bin3t = apool.tile([128, 2, CH, 10, 10], FP8, tag="bin3t")
        res4 = apool.tile([128, 4, CH, 4, 4], F32, tag="res4")
        bin4 = apool.tile([128, 4, CH, 6, 6], FP8, tag="bin4")
        bin4t = apool.tile([128, 4, CH, 6, 6], FP8, tag="bin4t")
        hsg = apool.tile([128, 4, CH, 16], BF16, tag="hsg")
        pooled = apool.tile([128, 4, B], F32, tag="pooled")

        for buf in (Rm, Rk, bin1, bin1t, bin2, bin2t, bin3, bin3t, bin4, bin4t):
            nc.vector.memset(buf[:], 0.0)

        def sgn(out_ap, in_ap, s=1.0, b=0.0):
            nc.scalar.activation(out_ap, in_ap, AT.Sign, bias=b, scale=s)

        def psum(npart, nfree):
            ps = ppool.tile([128, 512], F32, tag="ps")
            return ps[:npart, :nfree]

        for c in range(NCH):
            # ======== STEM + STAGE1 in half-chunks of SC images ========
            for sc in range(2):
                goff = c * CH + sc * SC          # global image offset in B
                loff = sc * SC                   # offset within chunk buffers
                gsl = slice(goff, goff + SC)
                for kh in range(7):
                    y0 = max(0, 3 - kh); y1 = min(31, 34 - kh)
                    sy0 = y0 + kh - 3
                    n_y = y1 - y0 + 1
                    for im in range(SC):
                        gim = goff + im
                        for arr, buf, ro in (('xa0', Rm, 0), ('xrh', Rk, 0),
                                             ('xrl', Rk, 21), ('xa08', Rk, 63),
                                             ('xa016', Rk, 84)):
                            nc.sync.dma_start(
                                buf[ro + 3 * kh:ro + 3 * kh + 3, im,
                                    y0:y1 + 1, 3:35],
                                d[arr][:, gim, sy0:sy0 + n_y, :])
                # duplicate blocks: a0 -> rows 42-62, xrh -> rows 105-125
                nc.sync.dma_start(Rk[42:63], Rm[:])
                nc.sync.dma_start(Rk[105:126], Rk[0:21])
                for img in range(SC):
                    bimg = loff + img            # index into bin1 (chunk-local)
                    for yh in range(2):
                        ys = 16 * yh
                        pv = psum(64, 512).rearrange("p (y x) -> p y x", y=16)
                        for kw in range(7):
                            nc.tensor.matmul(
                                pv[:], W['stwm'][:, kw, :],
                                Rm[:, img, ys:ys + 16, kw:kw + 32],
                                start=(kw == 0), stop=False)
                        for kw in range(7):
                            nc.tensor.matmul(
                                pv[:], W['stwa'][:, kw, :],
                                Rk[:, img, ys:ys + 16, kw:kw + 32],
                                start=False, stop=False)
                        for kw in range(7):
                            nc.tensor.matmul(
                                pv[:], W['stwb'][:, kw, :],
                                Rk[0:42, img, ys:ys + 16, kw:kw + 32],
                                start=False, stop=(kw == 6))
                        nc.vector.tensor_scalar(
                            res1[:, img, ys:ys + 16, :], pv[:],
                            W['bn1s'][:, 0:1], W['bn1t'][:, 0:1], OP.mult, OP.add)
                        sgn(bin1[:, bimg, 1 + ys:17 + ys, 1:33],
                            res1[:, img, ys:ys + 16, :])

                # -------- stage 1 (64ch 32x32), two blocks --------
                def conv3_64(pv, src, bimg, yh, widx):
                    i = 0
                    for kh in range(3):
                        for kw in range(3):
                            nc.tensor.matmul(
                                pv[:], W['s1w'][:, kh * 3 + kw, widx, :],
                                src[:, bimg, kh + 16 * yh:kh + 16 * yh + 16,
                                    kw:kw + 32],
                                start=(i == 0), stop=(i == 8))
                            i += 1
                for blk in range(2):
                    c1, c2 = 2 * blk, 2 * blk + 1
                    for img in range(SC):
                        bimg = loff + img
                        for yh in range(2):
                            pv = psum(64, 512).rearrange("p (y x) -> p y x", y=16)
                            conv3_64(pv, bin1, bimg, yh, c1)
                            sgn(bin1t[:, bimg, 1 + 16 * yh:17 + 16 * yh, 1:33],
                                pv[:], W['s1bs'][:, c1:c1 + 1],
                                W['s1bt'][:, c1:c1 + 1])
                    for img in range(SC):
                        bimg = loff + img
                        for yh in range(2):
                            ys = 16 * yh
                            pv = psum(64, 512).rearrange("p (y x) -> p y x", y=16)
                            conv3_64(pv, bin1t, bimg, yh, c2)
                            h2 = tpool.tile([64, 16, 32], F32, tag="h2a")
                            nc.vector.tensor_scalar(
                                h2[:], pv[:], W['s1bs'][:, c2:c2 + 1],
                                W['s1bt'][:, c2:c2 + 1], OP.mult, OP.add)
                            rsl = res1[:, img, ys:ys + 16, :]
                            nc.vector.tensor_tensor(rsl, h2[:], rsl, OP.add)
                            sgn(bin1[:, bimg, 1 + ys:17 + ys, 1:33], rsl)

            # ======== STAGES 2..4 on the full chunk ========
            def stage(si, cin, cout, get_prev, resp, binc, binct, H):
                kin = (cin + 127) // 128
                kout = (cout + 127) // 128
                px = CH * H * H
                ntile = max(1, px // 512)
                npt = CH // ntile
                Ws, Wt = W[f's{si}bs'], W[f's{si}bt']
                nfree = npt * H * H

                def pview(npart=128):
                    return psum(npart, nfree).rearrange(
                        "p (n y x) -> p n y x", n=npt, y=H)

                def srb(buf, g, ns2, kh, kw):   # stride-1 rhs from own stage buf
                    return buf[:, g, ns2, kh:kh + H, kw:kw + H]

                for tidx in range(ntile):
                    ns2 = slice(tidx * npt, (tidx + 1) * npt)
                    for m in range(kout):
                        mp = slice(128 * m, 128 * m + 128)
                        pv = pview()
                        i = 0
                        for g in range(kin):
                            for kh in range(3):
                                for kw in range(3):
                                    nc.tensor.matmul(
                                        pv[:], W[f's{si}c1w'][:, g, kh * 3 + kw, mp],
                                        get_prev(g, ns2, kh, kw, 2),
                                        start=(i == 0), stop=(i == kin * 9 - 1))
                                    i += 1
                        sgn(binct[:, m, ns2, 1:1 + H, 1:1 + H], pv[:],
                            Ws[:, m, 0:1], Wt[:, m, 0:1])
                        pvd = pview()
                        for g in range(kin):
                            nc.tensor.matmul(
                                pvd[:], W[f's{si}dw'][:, g, mp],
                                get_prev(g, ns2, 1, 1, 2),
                                start=(g == 0), stop=(g == kin - 1))
                        nc.vector.tensor_scalar(
                            resp[:, m, ns2, :, :], pvd[:],
                            Ws[:, m, 1:2], Wt[:, m, 1:2], OP.mult, OP.add)

                for cidx in range(3):
                    srcbuf = binct if cidx in (0, 2) else binc
                    dstbuf = binc if cidx in (0, 2) else binct
                    bnc = 2 + cidx
                    for tidx in range(ntile):
                        ns2 = slice(tidx * npt, (tidx + 1) * npt)
                        for m in range(kout):
                            mp = slice(128 * m, 128 * m + 128)
                            pv = pview()
                            i = 0
                            for g in range(kout):
                                for kh in range(3):
                                    for kw in range(3):
                                        nc.tensor.matmul(
                                            pv[:],
                                            W[f's{si}w'][:, g, kh * 3 + kw, cidx, mp],
                                            srb(srcbuf, g, ns2, kh, kw),
                                            start=(i == 0),
                                            stop=(i == kout * 9 - 1))
                                        i += 1
                            if cidx in (0, 2):
                                h2 = tpool.tile([128, 512], F32, tag="h2b")
                                h2v = h2[:, :nfree].rearrange(
                                    "p (n y x) -> p n y x", n=npt, y=H)
                                nc.vector.tensor_scalar(
                                    h2v[:], pv[:], Ws[:, m, bnc:bnc + 1],
                                    Wt[:, m, bnc:bnc + 1], OP.mult, OP.add)
                                rsl = resp[:, m, ns2, :, :]
                                nc.vector.tensor_tensor(rsl, h2v[:], rsl, OP.add)
                                sgn(dstbuf[:, m, ns2, 1:1 + H, 1:1 + H], rsl)
                            else:
                                sgn(dstbuf[:, m, ns2, 1:1 + H, 1:1 + H], pv[:],
                                    Ws[:, m, bnc:bnc + 1], Wt[:, m, bnc:bnc + 1])

            def prev1(g, ns2, kh, kw, stride):   # from bin1 (4D, chunk-local)
                return bin1[:, ns2, kh:kh + 31:2, kw:kw + 31:2]

            def prev2(g, ns2, kh, kw, stride):
                return bin2[:, 0, ns2, kh:kh + 15:2, kw:kw + 15:2]

            def prev3(g, ns2, kh, kw, stride):
                return bin3[:, g, ns2, kh:kh + 7:2, kw:kw + 7:2]

            stage(2, 64, 128, prev1, res2, bin2, bin2t, 16)
            stage(3, 128, 256, prev2, res3, bin3, bin3t, 8)
            stage(4, 256, 512, prev3, res4, bin4, bin4t, 4)

            # ======== POOL ========
            for m in range(4):
                sgn(hsg[:, m, :, :],
                    res4[:, m, :, :, :].rearrange("p n y x -> p n (y x)"))
            acc = tpool.tile([128, 4, CH], F32, tag="poolacc")
            nc.vector.tensor_copy(acc[:], hsg[:, :, :, 0])
            for i in range(1, 16):
                nc.vector.tensor_tensor(acc[:], acc[:], hsg[:, :, :, i], OP.add)
            nc.scalar.mul(pooled[:, :, c * CH:(c + 1) * CH], acc[:], 1.0 / 16.0)

        # ======== FC ========
        psf = pfc.tile([10, B], F32, tag="ps_fc")
        for g in range(4):
            nc.tensor.matmul(psf[:], W['fcw'][:, g, :], pooled[:, g, :],
                             start=(g == 0), stop=(g == 3))
        osb = tpool.tile([10, B], F32, tag="osb")
        nc.vector.tensor_scalar(osb[:], psf[:], W['fcb'][:, 0:1],
                                W['fcs'][:, 0:1], OP.add, OP.mult)
        nc.sync.dma_start(out_t[:], osb[:])
        ctx.close()

    _split_waits(nc)
    return nc


_NC_CACHE = None


def _get_nc():
    global _NC_CACHE
    if _NC_CACHE is None:
        _NC_CACHE = _build()
    return _NC_CACHE


def kernel(x, params, _trace=False):
    rep, shards = _prep_inputs(x, params)
    nc = _get_nc()
    in_maps = []
    for c in range(NCORES):
        m = dict(rep)
        m.update(shards[c])
        in_maps.append(m)
    res = run_bass_kernel_spmd(nc, in_maps, core_ids=list(range(NCORES)))
    outs = [r['out'].T for r in res.results]
    full = np.ascontiguousarray(np.concatenate(outs, axis=0), dtype=np.float32)
    if _trace:
        return full, res
    return full


# revision 11
# speedup vs baseline: 1.1499x; 1.0372x over previous
"""BitwiseResnet18 forward on 8 trn2 NeuronCores — pure batch data-parallel.

Numerics (matches jax-f32 CPU reference through every sign()):
- stem conv: exact-integer main term (a0*u0 on 8-bit grids in bf16 -> exact
  f32 PSUM integer accumulation) accumulated FIRST, then an algebraically
  exact f32 correction (a0*wr + xr*w) whose roundings are relative to the
  final value.  |err| <~ 1e-7, below the minimum sign margin (~1e-6).
- binarized convs: +-1 products in fp8, integer sums in f32 PSUM: exact.
- BN affines / residual adds: f32 mul-round-add-round on DVE, same as XLA.
"""
import copy
import numpy as np
import ml_dtypes

import concourse.bass as bass
import concourse.mybir as mybir
import concourse.tile as tile
from concourse.bass_utils import run_bass_kernel_spmd

NCORES = 8
B = 64          # per-core batch
CH = 8          # images per chunk (stages 2-4); stem/stage1 use CH//2
SC = CH // 2
F32 = mybir.dt.float32
BF16 = mybir.dt.bfloat16
FP8 = mybir.dt.float8e4
NP_FP8 = mybir.dt.np(FP8)
NP_BF16 = ml_dtypes.bfloat16

STAGES = [(64, 64, 1), (64, 128, 2), (128, 256, 2), (256, 512, 2)]


# --------------------------------------------------------------- wait splitter
def _split_waits(nc, max_waits=1):
    """The walrus build here accepts only one sync-wait per instruction;
    move extra waits onto injected EventSemaphore carriers."""
    mod = nc.m
    counter = [0]

    def carrier(engine, waits, debug):
        counter[0] += 1
        si = mybir.SyncInfo(on_wait=list(waits), on_update=[])
        return mybir.InstEventSemaphore(
            name=f"WSPLIT-{counter[0]}", engine=engine, sync_info=si,
            ins=[], outs=[], debug=debug)

    new_functions = []
    for function in mod.functions:
        nf = copy.replace(function, blocks=[])
        nf.set_allocations_from_list(function.allocations)
        for block in function.blocks:
            insts = []
            for inst in block.instructions:
                si = inst.sync_info
                waits = list(si.on_wait) if si is not None and si.on_wait else []
                if len(waits) > max_waits:
                    head, keep = waits[:-max_waits], waits[-max_waits:]
                    for i in range(0, len(head), max_waits):
                        insts.append(carrier(inst.engine, head[i:i + max_waits],
                                             inst.debug))
                    inst = copy.replace(inst, sync_info=mybir.SyncInfo(
                        on_wait=keep,
                        on_update=list(si.on_update) if si.on_update else []))
                insts.append(inst)
            nf.blocks.append(copy.replace(block, instructions=insts))
        new_functions.append(nf)
    new_mod = copy.replace(mod, functions=[])
    for f in new_functions:
        new_mod.functions.append(f)
    nc.m = new_mod
    return nc


# ------------------------------------------------------------------ host prep
def _f32(a):
    return np.asarray(a, dtype=np.float32)


def _bn_fold(p):
    var = _f32(p['var']); gamma = _f32(p['gamma'])
    beta = _f32(p['beta']); mean = _f32(p['mean'])
    inv = (np.float32(1.0) / np.sqrt(var + np.float32(1e-5))).astype(np.float32)
    s = (gamma * inv).astype(np.float32)
    t = (beta - (mean * gamma) * inv).astype(np.float32)
    return s, t


def _binw(w):
    w = np.asarray(w, dtype=np.float32)
    sw = np.where(w >= 0, np.float32(1), np.float32(-1))
    lhsT = np.transpose(sw, (1, 2, 3, 0)).reshape(sw.shape[1], 9, sw.shape[0])
    return np.ascontiguousarray(lhsT).astype(NP_FP8)          # [Cin, 9, Cout]


def _binw1x1(w):
    w = np.asarray(w, dtype=np.float32)[:, :, 0, 0]
    sw = np.where(w >= 0, np.float32(1), np.float32(-1))
    return np.ascontiguousarray(sw.T).astype(NP_FP8)          # [Cin, Cout]


def _prep_inputs(x, params):
    x = np.asarray(x, dtype=np.float32)
    x64 = x.astype(np.float64)
    a0 = np.round(x64 * 32.0) / 32.0          # 2^-5 grid, ints <= 173
    resid = x64 - a0
    xrh_s = (resid * 2.0 ** 12).astype(np.float16)       # scaled limb
    xrh = xrh_s.astype(np.float64) * 2.0 ** -12
    xrl_s = ((resid - xrh) * 2.0 ** 24).astype(np.float16)

    w64 = np.asarray(params['conv1_w'], np.float32).astype(np.float64)
    u0 = np.round(w64 * 256.0) / 256.0        # 2^-8 grid, ints <= 167 (bf16 ok)
    wr = w64 - u0
    wr1 = np.round(wr * 2.0 ** 16) * 2.0 ** -16
    wr2 = np.round((wr - wr1) * 2.0 ** 24) * 2.0 ** -24
    wr3 = np.round((wr - wr1 - wr2) * 2.0 ** 32) * 2.0 ** -32
    um = np.zeros((21, 7, 64), np.float64)    # [row, kw, Cout]
    wa = np.zeros((126, 7, 64), np.float64)
    wb = np.zeros((42, 7, 64), np.float64)
    for kw in range(7):
        for kh in range(7):
            for ci in range(3):
                r = kh * 3 + ci
                um[r, kw, :] = u0[:, ci, kh, kw]
                wa[r, kw, :] = u0[:, ci, kh, kw] * 2.0 ** -12      # x xrh_s
                wa[21 + r, kw, :] = u0[:, ci, kh, kw] * 2.0 ** -24  # x xrl_s
                wa[42 + r, kw, :] = wr1[:, ci, kh, kw]              # x a0
                wa[63 + r, kw, :] = wr2[:, ci, kh, kw] * 2.0 ** 8   # x a0*2^-8
                wa[84 + r, kw, :] = wr3[:, ci, kh, kw] * 2.0 ** 16  # x a0*2^-16
                wa[105 + r, kw, :] = wr1[:, ci, kh, kw] * 2.0 ** -12  # x xrh_s
                wb[r, kw, :] = wr2[:, ci, kh, kw] * 2.0 ** -12      # x xrh_s
                wb[21 + r, kw, :] = wr1[:, ci, kh, kw] * 2.0 ** -24  # x xrl_s
    rep = {'stwm': um.astype(NP_BF16), 'stwa': wa.astype(NP_BF16),
           'stwb': wb.astype(NP_BF16)}

    s, t = _bn_fold(params['bn1'])
    rep['bn1s'] = s.reshape(64, 1)
    rep['bn1t'] = t.reshape(64, 1)

    st = params['stages']
    w1 = np.stack([_binw(st[0][0]['conv1_w']), _binw(st[0][0]['conv2_w']),
                   _binw(st[0][1]['conv1_w']), _binw(st[0][1]['conv2_w'])], axis=2)
    rep['s1w'] = np.ascontiguousarray(w1)                     # [64, 9, 4, 64]
    bs = [_bn_fold(st[0][0]['bn1']), _bn_fold(st[0][0]['bn2']),
          _bn_fold(st[0][1]['bn1']), _bn_fold(st[0][1]['bn2'])]
    rep['s1bs'] = np.ascontiguousarray(np.stack([b[0] for b in bs], 1))
    rep['s1bt'] = np.ascontiguousarray(np.stack([b[1] for b in bs], 1))

    for si, (cin, cout, stride) in enumerate(STAGES[1:], start=2):
        b1, b2 = st[si - 1]
        kin = (cin + 127) // 128
        kout = (cout + 127) // 128
        c1 = _binw(b1['conv1_w'])
        dw = _binw1x1(b1['down_w'])
        rest = np.stack([_binw(b1['conv2_w']), _binw(b2['conv1_w']),
                         _binw(b2['conv2_w'])], axis=2)       # [cout,9,3,cout]
        kp = min(cin, 128)
        rep[f's{si}c1w'] = np.ascontiguousarray(
            c1.reshape(kin, kp, 9, cout).transpose(1, 0, 2, 3))
        rep[f's{si}dw'] = np.ascontiguousarray(
            dw.reshape(kin, kp, cout).transpose(1, 0, 2))
        rep[f's{si}w'] = np.ascontiguousarray(
            rest.reshape(kout, 128, 9, 3, cout).transpose(1, 0, 2, 3, 4))
        bs = [_bn_fold(b1['bn1']), _bn_fold(b1['down_bn']), _bn_fold(b1['bn2']),
              _bn_fold(b2['bn1']), _bn_fold(b2['bn2'])]
        rep[f's{si}bs'] = np.ascontiguousarray(
            np.stack([b[0] for b in bs], 1).reshape(kout, 128, 5).transpose(1, 0, 2))
        rep[f's{si}bt'] = np.ascontiguousarray(
            np.stack([b[1] for b in bs], 1).reshape(kout, 128, 5).transpose(1, 0, 2))

    fcw = _f32(params['fc_w'])
    rep['fcw'] = np.ascontiguousarray(fcw.T.reshape(4, 128, 10).transpose(1, 0, 2))
    rep['fcb'] = _f32(params['fc_b']).reshape(10, 1)
    rep['fcs'] = _f32(params['scale']).reshape(10, 1)

    shards = []
    a0h = a0.astype(np.float16)
    a08 = (a0 * 2.0 ** -8).astype(np.float16)
    a016 = (a0 * 2.0 ** -16).astype(np.float16)
    for c in range(NCORES):
        sl = slice(c * B, (c + 1) * B)
        shards.append({
            'xa0': np.ascontiguousarray(a0h[sl].transpose(1, 0, 2, 3)),
            'xa08': np.ascontiguousarray(a08[sl].transpose(1, 0, 2, 3)),
            'xa016': np.ascontiguousarray(a016[sl].transpose(1, 0, 2, 3)),
            'xrh': np.ascontiguousarray(xrh_s[sl].transpose(1, 0, 2, 3)),
            'xrl': np.ascontiguousarray(xrl_s[sl].transpose(1, 0, 2, 3)),
        })
    return rep, shards


# ------------------------------------------------------------------ bass build
def _build():
    nc = bass.Bass()
    NCH = B // CH
    AT = mybir.ActivationFunctionType
    OP = mybir.AluOpType

    d = {}
    FP16 = mybir.dt.float16
    for nm in ('xa0', 'xa08', 'xa016', 'xrh', 'xrl'):
        d[nm] = nc.dram_tensor(nm, [3, B, 32, 32], FP16, kind="ExternalInput")
    d['stwm'] = nc.dram_tensor("stwm", [21, 7, 64], BF16, kind="ExternalInput")
    d['stwa'] = nc.dram_tensor("stwa", [126, 7, 64], BF16, kind="ExternalInput")
    d['stwb'] = nc.dram_tensor("stwb", [42, 7, 64], BF16, kind="ExternalInput")
    d['bn1s'] = nc.dram_tensor("bn1s", [64, 1], F32, kind="ExternalInput")
    d['bn1t'] = nc.dram_tensor("bn1t", [64, 1], F32, kind="ExternalInput")
    d['s1w'] = nc.dram_tensor("s1w", [64, 9, 4, 64], FP8, kind="ExternalInput")
    d['s1bs'] = nc.dram_tensor("s1bs", [64, 4], F32, kind="ExternalInput")
    d['s1bt'] = nc.dram_tensor("s1bt", [64, 4], F32, kind="ExternalInput")
    for si, (cin, cout, stride) in enumerate(STAGES[1:], start=2):
        kin = (cin + 127) // 128
        kout = (cout + 127) // 128
        kp = min(cin, 128)
        d[f's{si}c1w'] = nc.dram_tensor(f"s{si}c1w", [kp, kin, 9, cout], FP8, kind="ExternalInput")
        d[f's{si}dw'] = nc.dram_tensor(f"s{si}dw", [kp, kin, cout], FP8, kind="ExternalInput")
        d[f's{si}w'] = nc.dram_tensor(f"s{si}w", [128, kout, 9, 3, cout], FP8, kind="ExternalInput")
        d[f's{si}bs'] = nc.dram_tensor(f"s{si}bs", [128, kout, 5], F32, kind="ExternalInput")
        d[f's{si}bt'] = nc.dram_tensor(f"s{si}bt", [128, kout, 5], F32, kind="ExternalInput")
    d['fcw'] = nc.dram_tensor("fcw", [128, 4, 10], F32, kind="ExternalInput")
    d['fcb'] = nc.dram_tensor("fcb", [10, 1], F32, kind="ExternalInput")
    d['fcs'] = nc.dram_tensor("fcs", [10, 1], F32, kind="ExternalInput")
    out_t = nc.dram_tensor("out", [10, B], F32, kind="ExternalOutput")

    with tile.TileContext(nc) as tc:
        import contextlib
        ctx = contextlib.ExitStack()
        wpool = ctx.enter_context(tc.tile_pool(name="w", bufs=1))
        apool = ctx.enter_context(tc.tile_pool(name="a", bufs=1))
        tpool = ctx.enter_context(tc.tile_pool(name="t", bufs=2))
        ppool = ctx.enter_context(tc.tile_pool(name="p", bufs=6, space="PSUM"))
        pfc = ctx.enter_context(tc.tile_pool(name="pfc", bufs=1, space="PSUM"))

        W = {}
        for k in d:
            if k in ('xa0', 'xa08', 'xa016', 'xrh', 'xrl'):
                continue
            W[k] = wpool.tile(list(d[k].shape), d[k].dtype, tag=k, name=k)
            nc.sync.dma_start(W[k][:], d[k][:])

        rpool = ctx.enter_context(tc.tile_pool(name="r", bufs=2))
        res1 = apool.tile([64, SC, 32, 32], F32, tag="res1")
        bin1 = apool.tile([64, CH, 34, 34], FP8, tag="bin1")
        bin1t = apool.tile([64, CH, 34, 34], FP8, tag="bin1t")
        res2 = apool.tile([128, 1, CH, 16, 16], F32, tag="res2")
        bin2 = apool.tile([128, 1, CH, 18, 18], FP8, tag="bin2")
        bin2t = apool.tile([128, 1, CH, 18, 18], FP8, tag="bin2t")
        res3 = apool.tile([128, 2, CH, 8, 8], F32, tag="res3")
        bin3 = apool.tile([128, 2, CH, 10, 10], FP8, tag="bin3")
        bin3t = apool.tile([128, 2, CH, 10, 10], FP8, tag="bin3t")
        res4 = apool.tile([128, 4, CH, 4, 4], F32, tag="res4")
        bin4 = apool.tile([128, 4, CH, 6, 6], FP8, tag="bin4")
        bin4t = apool.tile([128, 4, CH, 6, 6], FP8, tag="bin4t")
        hsg = apool.tile([128, 4, CH, 16], BF16, tag="hsg")
        pooled = apool.tile([128, 4, B], F32, tag="pooled")

        for buf in (bin1, bin1t, bin2, bin2t, bin3, bin3t, bin4, bin4t):
            nc.vector.memset(buf[:], 0.0)

        def sgn(out_ap, in_ap, s=1.0, b=0.0):
            nc.scalar.activation(out_ap, in_ap, AT.Sign, bias=b, scale=s)

        def psum(npart, nfree):
            ps = ppool.tile([128, 512], F32, tag="ps")
            return ps[:npart, :nfree]

        for c in range(NCH):
            # ======== STEM + STAGE1 in half-chunks of SC images ========
            for sc in range(2):
                goff = c * CH + sc * SC          # global image offset in B
                loff = sc * SC                   # offset within chunk buffers
                gsl = slice(goff, goff + SC)
                Rm = rpool.tile([21, SC, 32, 38], FP16, tag="Rm")
                Rk = rpool.tile([126, SC, 32, 38], FP16, tag="Rk")
                nc.vector.memset(Rm[:], 0.0)
                nc.vector.memset(Rk[:], 0.0)
                for kh in range(7):
                    y0 = max(0, 3 - kh); y1 = min(31, 34 - kh)
                    sy0 = y0 + kh - 3
                    n_y = y1 - y0 + 1
                    for im in range(SC):
                        gim = goff + im
                        for arr, buf, ro in (('xa0', Rm, 0), ('xrh', Rk, 0),
                                             ('xrl', Rk, 21), ('xa08', Rk, 63),
                                             ('xa016', Rk, 84)):
                            nc.sync.dma_start(
                                buf[ro + 3 * kh:ro + 3 * kh + 3, im,
                                    y0:y1 + 1, 3:35],
                                d[arr][:, gim, sy0:sy0 + n_y, :])
                # duplicate blocks: a0 -> rows 42-62, xrh -> rows 105-125
                nc.sync.dma_start(Rk[42:63], Rm[:])
                nc.sync.dma_start(Rk[105:126], Rk[0:21])
                for img in range(SC):
                    bimg = loff + img            # index into bin1 (chunk-local)
                    for yh in range(2):
                        ys = 16 * yh
                        pv = psum(64, 512).rearrange("p (y x) -> p y x", y=16)
                        for kw in range(7):
                            nc.tensor.matmul(
                                pv[:], W['stwm'][:, kw, :],
                                Rm[:, img, ys:ys + 16, kw:kw + 32],
                                start=(kw == 0), stop=False)
                        for kw in range(7):
                            nc.tensor.matmul(
                                pv[:], W['stwa'][:, kw, :],
                                Rk[:, img, ys:ys + 16, kw:kw + 32],
                                start=False, stop=False)
                        for kw in range(7):
                            nc.tensor.matmul(
                                pv[:], W['stwb'][:, kw, :],
                                Rk[0:42, img, ys:ys + 16, kw:kw + 32],
                                start=False, stop=(kw == 6))
                        nc.vector.tensor_scalar(
                            res1[:, img, ys:ys + 16, :], pv[:],
                            W['bn1s'][:, 0:1], W['bn1t'][:, 0:1], OP.mult, OP.add)
                        sgn(bin1[:, bimg, 1 + ys:17 + ys, 1:33],
                            res1[:, img, ys:ys + 16, :])

                # -------- stage 1 (64ch 32x32), two blocks --------
                def conv3_64(pv, src, bimg, yh, widx):
                    i = 0
                    for kh in range(3):
                        for kw in range(3):
                            nc.tensor.matmul(
                                pv[:], W['s1w'][:, kh * 3 + kw, widx, :],
                                src[:, bimg, kh + 16 * yh:kh + 16 * yh + 16,
                                    kw:kw + 32],
                                start=(i == 0), stop=(i == 8))
                            i += 1
                for blk in range(2):
                    c1, c2 = 2 * blk, 2 * blk + 1
                    for img in range(SC):
                        bimg = loff + img
                        for yh in range(2):
                            pv = psum(64, 512).rearrange("p (y x) -> p y x", y=16)
                            conv3_64(pv, bin1, bimg, yh, c1)
                            sgn(bin1t[:, bimg, 1 + 16 * yh:17 + 16 * yh, 1:33],
                                pv[:], W['s1bs'][:, c1:c1 + 1],
                                W['s1bt'][:, c1:c1 + 1])
                    for img in range(SC):
                        bimg = loff + img
                        for yh in range(2):
                            ys = 16 * yh
                            pv = psum(64, 512).rearrange("p (y x) -> p y x", y=16)
                            conv3_64(pv, bin1t, bimg, yh, c2)
                            h2 = tpool.tile([64, 16, 32], F32, tag="h2a")
                            nc.vector.tensor_scalar(
                                h2[:], pv[:], W['s1bs'][:, c2:c2 + 1],
                                W['s1bt'][:, c2:c2 + 1], OP.mult, OP.add)
                            rsl = res1[:, img, ys:ys + 16, :]
                            nc.vector.tensor_tensor(rsl, h2[:], rsl, OP.add)
                            sgn(bin1[:, bimg, 1 + ys:17 + ys, 1:33], rsl)

            # ======== STAGES 2..4 on the full chunk ========
            def stage(si, cin, cout, get_prev, resp, binc, binct, H):
                kin = (cin + 127) // 128
                kout = (cout + 127) // 128
                px = CH * H * H
                ntile = max(1, px // 512)
                npt = CH // ntile
                Ws, Wt = W[f's{si}bs'], W[f's{si}bt']
                nfree = npt * H * H

                def pview(npart=128):
                    return psum(npart, nfree).rearrange(
                        "p (n y x) -> p n y x", n=npt, y=H)

                def srb(buf, g, ns2, kh, kw):   # stride-1 rhs from own stage buf
                    return buf[:, g, ns2, kh:kh + H, kw:kw + H]

                for tidx in range(ntile):
                    ns2 = slice(tidx * npt, (tidx + 1) * npt)
                    for m in range(kout):
                        mp = slice(128 * m, 128 * m + 128)
                        pv = pview()
                        i = 0
                        for g in range(kin):
                            for kh in range(3):
                                for kw in range(3):
                                    nc.tensor.matmul(
                                        pv[:], W[f's{si}c1w'][:, g, kh * 3 + kw, mp],
                                        get_prev(g, ns2, kh, kw, 2),
                                        start=(i == 0), stop=(i == kin * 9 - 1))
                                    i += 1
                        sgn(binct[:, m, ns2, 1:1 + H, 1:1 + H], pv[:],
                            Ws[:, m, 0:1], Wt[:, m, 0:1])
                        pvd = pview()
                        for g in range(kin):
                            nc.tensor.matmul(
                                pvd[:], W[f's{si}dw'][:, g, mp],
                                get_prev(g, ns2, 1, 1, 2),
                                start=(g == 0), stop=(g == kin - 1))
                        nc.vector.tensor_scalar(
                            resp[:, m, ns2, :, :], pvd[:],
                            Ws[:, m, 1:2], Wt[:, m, 1:2], OP.mult, OP.add)

                for cidx in range(3):
                    srcbuf = binct if cidx in (0, 2) else binc
                    dstbuf = binc if cidx in (0, 2) else binct
                    bnc = 2 + cidx
                    for tidx in range(ntile):
                        ns2 = slice(tidx * npt, (tidx + 1) * npt)
                        for m in range(kout):
                            mp = slice(128 * m, 128 * m + 128)
                            pv = pview()
                            i = 0
                            for g in range(kout):
                                for kh in range(3):
                                    for kw in range(3):
                                        nc.tensor.matmul(
                                            pv[:],
                                            W[f's{si}w'][:, g, kh * 3 + kw, cidx, mp],
                                            srb(srcbuf, g, ns2, kh, kw),
                                            start=(i == 0),
                                            stop=(i == kout * 9 - 1))
                                        i += 1
                            if cidx in (0, 2):
                                h2 = tpool.tile([128, 512], F32, tag="h2b")
                                h2v = h2[:, :nfree].rearrange(
                                    "p (n y x) -> p n y x", n=npt, y=H)
                                nc.vector.tensor_scalar(
                                    h2v[:], pv[:], Ws[:, m, bnc:bnc + 1],
                                    Wt[:, m, bnc:bnc + 1], OP.mult, OP.add)
                                rsl = resp[:, m, ns2, :, :]
                                nc.vector.tensor_tensor(rsl, h2v[:], rsl, OP.add)
                                sgn(dstbuf[:, m, ns2, 1:1 + H, 1:1 + H], rsl)
                            else:
                                sgn(dstbuf[:, m, ns2, 1:1 + H, 1:1 + H], pv[:],
                                    Ws[:, m, bnc:bnc + 1], Wt[:, m, bnc:bnc + 1])

            def prev1(g, ns2, kh, kw, stride):   # from bin1 (4D, chunk-local)
                return bin1[:, ns2, kh:kh + 31:2, kw:kw + 31:2]

            def prev2(g, ns2, kh, kw, stride):
                return bin2[:, 0, ns2, kh:kh + 15:2, kw:kw + 15:2]

            def prev3(g, ns2, kh, kw, stride):
                return bin3[:, g, ns2, kh:kh + 7:2, kw:kw + 7:2]

            stage(2, 64, 128, prev1, res2, bin2, bin2t, 16)
            stage(3, 128, 256, prev2, res3, bin3, bin3t, 8)
            stage(4, 256, 512, prev3, res4, bin4, bin4t, 4)

            # ======== POOL ========
            for m in range(4):
                sgn(hsg[:, m, :, :],
                    res4[:, m, :, :, :].rearrange("p n y x -> p n (y x)"))
            acc = tpool.tile([128, 4, CH], F32, tag="poolacc")
            nc.vector.tensor_copy(acc[:], hsg[:, :, :, 0])
            for i in range(1, 16):
                nc.vector.tensor_tensor(acc[:], acc[:], hsg[:, :, :, i], OP.add)
            nc.scalar.mul(pooled[:, :, c * CH:(c + 1) * CH], acc[:], 1.0 / 16.0)

        # ======== FC ========
        psf = pfc.tile([10, B], F32, tag="ps_fc")
        for g in range(4):
            nc.tensor.matmul(psf[:], W['fcw'][:, g, :], pooled[:, g, :],
                             start=(g == 0), stop=(g == 3))
        osb = tpool.tile([10, B], F32, tag="osb")
        nc.vector.tensor_scalar(osb[:], psf[:], W['fcb'][:, 0:1],
                                W['fcs'][:, 0:1], OP.add, OP.mult)
        nc.sync.dma_start(out_t[:], osb[:])
        ctx.close()

    _split_waits(nc)
    return nc


_NC_CACHE = None


def _get_nc():
    global _NC_CACHE
    if _NC_CACHE is None:
        _NC_CACHE = _build()
    return _NC_CACHE


def kernel(x, params, _trace=False):
    rep, shards = _prep_inputs(x, params)
    nc = _get_nc()
    in_maps = []
    for c in range(NCORES):
        m = dict(rep)
        m.update(shards[c])
        in_maps.append(m)
    res = run_bass_kernel_spmd(nc, in_maps, core_ids=list(range(NCORES)))
    outs = [r['out'].T for r in res.results]
    full = np.ascontiguousarray(np.concatenate(outs, axis=0), dtype=np.float32)
    if _trace:
        return full, res
    return full


# revision 12
# speedup vs baseline: 1.2087x; 1.0512x over previous
"""BitwiseResnet18 forward on 8 trn2 NeuronCores — pure batch data-parallel.

Numerics (matches jax-f32 CPU reference through every sign()):
- stem conv: exact-integer main term (a0*u0 on 8-bit grids in bf16 -> exact
  f32 PSUM integer accumulation) accumulated FIRST, then an algebraically
  exact f32 correction (a0*wr + xr*w) whose roundings are relative to the
  final value.  |err| <~ 1e-7, below the minimum sign margin (~1e-6).
- binarized convs: +-1 products in fp8, integer sums in f32 PSUM: exact.
- BN affines / residual adds: f32 mul-round-add-round on DVE, same as XLA.
"""
import copy
import numpy as np
import ml_dtypes

import concourse.bass as bass
import concourse.mybir as mybir
import concourse.tile as tile
from concourse.bass_utils import run_bass_kernel_spmd

NCORES = 8
B = 64          # per-core batch
CH = 8          # images per chunk (stages 2-4); stem/stage1 use CH//2
SC = CH // 2
F32 = mybir.dt.float32
BF16 = mybir.dt.bfloat16
FP8 = mybir.dt.float8e4
NP_FP8 = mybir.dt.np(FP8)
NP_BF16 = ml_dtypes.bfloat16

STAGES = [(64, 64, 1), (64, 128, 2), (128, 256, 2), (256, 512, 2)]


# --------------------------------------------------------------- wait splitter
def _split_waits(nc, max_waits=1):
    """The walrus build here accepts only one sync-wait per instruction;
    move extra waits onto injected EventSemaphore carriers."""
    mod = nc.m
    counter = [0]

    def carrier(engine, waits, debug):
        counter[0] += 1
        si = mybir.SyncInfo(on_wait=list(waits), on_update=[])
        return mybir.InstEventSemaphore(
            name=f"WSPLIT-{counter[0]}", engine=engine, sync_info=si,
            ins=[], outs=[], debug=debug)

    new_functions = []
    for function in mod.functions:
        nf = copy.replace(function, blocks=[])
        nf.set_allocations_from_list(function.allocations)
        for block in function.blocks:
            insts = []
            for inst in block.instructions:
                si = inst.sync_info
                waits = list(si.on_wait) if si is not None and si.on_wait else []
                if len(waits) > max_waits:
                    head, keep = waits[:-max_waits], waits[-max_waits:]
                    for i in range(0, len(head), max_waits):
                        insts.append(carrier(inst.engine, head[i:i + max_waits],
                                             inst.debug))
                    inst = copy.replace(inst, sync_info=mybir.SyncInfo(
                        on_wait=keep,
                        on_update=list(si.on_update) if si.on_update else []))
                insts.append(inst)
            nf.blocks.append(copy.replace(block, instructions=insts))
        new_functions.append(nf)
    new_mod = copy.replace(mod, functions=[])
    for f in new_functions:
        new_mod.functions.append(f)
    nc.m = new_mod
    return nc


# ------------------------------------------------------------------ host prep
def _f32(a):
    return np.asarray(a, dtype=np.float32)


def _bn_fold(p):
    var = _f32(p['var']); gamma = _f32(p['gamma'])
    beta = _f32(p['beta']); mean = _f32(p['mean'])
    inv = (np.float32(1.0) / np.sqrt(var + np.float32(1e-5))).astype(np.float32)
    s = (gamma * inv).astype(np.float32)
    t = (beta - (mean * gamma) * inv).astype(np.float32)
    return s, t


def _binw(w):
    w = np.asarray(w, dtype=np.float32)
    sw = np.where(w >= 0, np.float32(1), np.float32(-1))
    lhsT = np.transpose(sw, (1, 2, 3, 0)).reshape(sw.shape[1], 9, sw.shape[0])
    return np.ascontiguousarray(lhsT).astype(NP_FP8)          # [Cin, 9, Cout]


def _binw1x1(w):
    w = np.asarray(w, dtype=np.float32)[:, :, 0, 0]
    sw = np.where(w >= 0, np.float32(1), np.float32(-1))
    return np.ascontiguousarray(sw.T).astype(NP_FP8)          # [Cin, Cout]


def _prep_inputs(x, params):
    x = np.asarray(x, dtype=np.float32)
    x64 = x.astype(np.float64)
    a0 = np.round(x64 * 32.0) / 32.0          # 2^-5 grid, ints <= 173
    resid = x64 - a0
    xrh_s = (resid * 2.0 ** 12).astype(np.float16)       # scaled limb
    xrh = xrh_s.astype(np.float64) * 2.0 ** -12
    xrl_s = ((resid - xrh) * 2.0 ** 24).astype(np.float16)

    w64 = np.asarray(params['conv1_w'], np.float32).astype(np.float64)
    u0 = np.round(w64 * 256.0) / 256.0        # 2^-8 grid, ints <= 167 (bf16 ok)
    wr = w64 - u0
    wr1 = np.round(wr * 2.0 ** 16) * 2.0 ** -16
    wr2 = np.round((wr - wr1) * 2.0 ** 24) * 2.0 ** -24
    wr3 = np.round((wr - wr1 - wr2) * 2.0 ** 32) * 2.0 ** -32
    um = np.zeros((21, 7, 64), np.float64)    # [row, kw, Cout]
    wa = np.zeros((126, 7, 64), np.float64)
    wb = np.zeros((42, 7, 64), np.float64)
    for kw in range(7):
        for kh in range(7):
            for ci in range(3):
                r = kh * 3 + ci
                um[r, kw, :] = u0[:, ci, kh, kw]
                wa[r, kw, :] = u0[:, ci, kh, kw] * 2.0 ** -12      # x xrh_s
                wa[21 + r, kw, :] = u0[:, ci, kh, kw] * 2.0 ** -24  # x xrl_s
                wa[42 + r, kw, :] = wr1[:, ci, kh, kw]              # x a0
                wa[63 + r, kw, :] = wr2[:, ci, kh, kw] * 2.0 ** 8   # x a0*2^-8
                wa[84 + r, kw, :] = wr3[:, ci, kh, kw] * 2.0 ** 16  # x a0*2^-16
                wa[105 + r, kw, :] = wr1[:, ci, kh, kw] * 2.0 ** -12  # x xrh_s
                wb[r, kw, :] = wr2[:, ci, kh, kw] * 2.0 ** -12      # x xrh_s
                wb[21 + r, kw, :] = wr1[:, ci, kh, kw] * 2.0 ** -24  # x xrl_s
    um6 = np.concatenate([um[:, kw, :] for kw in range(6)], axis=0)  # [126, 64]
    rep = {'stwm': um.astype(NP_BF16), 'stwm6': um6.reshape(126, 1, 64).astype(NP_BF16),
           'stwa': wa.astype(NP_BF16), 'stwb': wb.astype(NP_BF16)}

    s, t = _bn_fold(params['bn1'])
    rep['bn1s'] = s.reshape(64, 1)
    rep['bn1t'] = t.reshape(64, 1)

    st = params['stages']
    w1 = np.stack([_binw(st[0][0]['conv1_w']), _binw(st[0][0]['conv2_w']),
                   _binw(st[0][1]['conv1_w']), _binw(st[0][1]['conv2_w'])], axis=2)
    rep['s1w'] = np.ascontiguousarray(w1)                     # [64, 9, 4, 64]
    bs = [_bn_fold(st[0][0]['bn1']), _bn_fold(st[0][0]['bn2']),
          _bn_fold(st[0][1]['bn1']), _bn_fold(st[0][1]['bn2'])]
    rep['s1bs'] = np.ascontiguousarray(np.stack([b[0] for b in bs], 1))
    rep['s1bt'] = np.ascontiguousarray(np.stack([b[1] for b in bs], 1))

    for si, (cin, cout, stride) in enumerate(STAGES[1:], start=2):
        b1, b2 = st[si - 1]
        kin = (cin + 127) // 128
        kout = (cout + 127) // 128
        c1 = _binw(b1['conv1_w'])
        dw = _binw1x1(b1['down_w'])
        rest = np.stack([_binw(b1['conv2_w']), _binw(b2['conv1_w']),
                         _binw(b2['conv2_w'])], axis=2)       # [cout,9,3,cout]
        kp = min(cin, 128)
        rep[f's{si}c1w'] = np.ascontiguousarray(
            c1.reshape(kin, kp, 9, cout).transpose(1, 0, 2, 3))
        rep[f's{si}dw'] = np.ascontiguousarray(
            dw.reshape(kin, kp, cout).transpose(1, 0, 2))
        rep[f's{si}w'] = np.ascontiguousarray(
            rest.reshape(kout, 128, 9, 3, cout).transpose(1, 0, 2, 3, 4))
        bs = [_bn_fold(b1['bn1']), _bn_fold(b1['down_bn']), _bn_fold(b1['bn2']),
              _bn_fold(b2['bn1']), _bn_fold(b2['bn2'])]
        rep[f's{si}bs'] = np.ascontiguousarray(
            np.stack([b[0] for b in bs], 1).reshape(kout, 128, 5).transpose(1, 0, 2))
        rep[f's{si}bt'] = np.ascontiguousarray(
            np.stack([b[1] for b in bs], 1).reshape(kout, 128, 5).transpose(1, 0, 2))

    fcw = _f32(params['fc_w'])
    rep['fcw'] = np.ascontiguousarray(fcw.T.reshape(4, 128, 10).transpose(1, 0, 2))
    rep['fcb'] = _f32(params['fc_b']).reshape(10, 1)
    rep['fcs'] = _f32(params['scale']).reshape(10, 1)

    shards = []
    a0h = a0.astype(np.float16)
    a08 = (a0 * 2.0 ** -8).astype(np.float16)
    a016 = (a0 * 2.0 ** -16).astype(np.float16)
    for c in range(NCORES):
        sl = slice(c * B, (c + 1) * B)
        shards.append({
            'xa0': np.ascontiguousarray(a0h[sl].transpose(1, 0, 2, 3)),
            'xa08': np.ascontiguousarray(a08[sl].transpose(1, 0, 2, 3)),
            'xa016': np.ascontiguousarray(a016[sl].transpose(1, 0, 2, 3)),
            'xrh': np.ascontiguousarray(xrh_s[sl].transpose(1, 0, 2, 3)),
            'xrl': np.ascontiguousarray(xrl_s[sl].transpose(1, 0, 2, 3)),
        })
    return rep, shards


# ------------------------------------------------------------------ bass build
def _build():
    nc = bass.Bass()
    NCH = B // CH
    AT = mybir.ActivationFunctionType
    OP = mybir.AluOpType

    d = {}
    FP16 = mybir.dt.float16
    for nm in ('xa0', 'xa08', 'xa016', 'xrh', 'xrl'):
        d[nm] = nc.dram_tensor(nm, [3, B, 32, 32], FP16, kind="ExternalInput")
    d['stwm'] = nc.dram_tensor("stwm", [21, 7, 64], BF16, kind="ExternalInput")
    d['stwm6'] = nc.dram_tensor("stwm6", [126, 1, 64], BF16, kind="ExternalInput")
    d['stwa'] = nc.dram_tensor("stwa", [126, 7, 64], BF16, kind="ExternalInput")
    d['stwb'] = nc.dram_tensor("stwb", [42, 7, 64], BF16, kind="ExternalInput")
    d['bn1s'] = nc.dram_tensor("bn1s", [64, 1], F32, kind="ExternalInput")
    d['bn1t'] = nc.dram_tensor("bn1t", [64, 1], F32, kind="ExternalInput")
    d['s1w'] = nc.dram_tensor("s1w", [64, 9, 4, 64], FP8, kind="ExternalInput")
    d['s1bs'] = nc.dram_tensor("s1bs", [64, 4], F32, kind="ExternalInput")
    d['s1bt'] = nc.dram_tensor("s1bt", [64, 4], F32, kind="ExternalInput")
    for si, (cin, cout, stride) in enumerate(STAGES[1:], start=2):
        kin = (cin + 127) // 128
        kout = (cout + 127) // 128
        kp = min(cin, 128)
        d[f's{si}c1w'] = nc.dram_tensor(f"s{si}c1w", [kp, kin, 9, cout], FP8, kind="ExternalInput")
        d[f's{si}dw'] = nc.dram_tensor(f"s{si}dw", [kp, kin, cout], FP8, kind="ExternalInput")
        d[f's{si}w'] = nc.dram_tensor(f"s{si}w", [128, kout, 9, 3, cout], FP8, kind="ExternalInput")
        d[f's{si}bs'] = nc.dram_tensor(f"s{si}bs", [128, kout, 5], F32, kind="ExternalInput")
        d[f's{si}bt'] = nc.dram_tensor(f"s{si}bt", [128, kout, 5], F32, kind="ExternalInput")
    d['fcw'] = nc.dram_tensor("fcw", [128, 4, 10], F32, kind="ExternalInput")
    d['fcb'] = nc.dram_tensor("fcb", [10, 1], F32, kind="ExternalInput")
    d['fcs'] = nc.dram_tensor("fcs", [10, 1], F32, kind="ExternalInput")
    out_t = nc.dram_tensor("out", [10, B], F32, kind="ExternalOutput")

    with tile.TileContext(nc) as tc:
        import contextlib
        ctx = contextlib.ExitStack()
        wpool = ctx.enter_context(tc.tile_pool(name="w", bufs=1))
        apool = ctx.enter_context(tc.tile_pool(name="a", bufs=1))
        tpool = ctx.enter_context(tc.tile_pool(name="t", bufs=2))
        ppool = ctx.enter_context(tc.tile_pool(name="p", bufs=6, space="PSUM"))
        pfc = ctx.enter_context(tc.tile_pool(name="pfc", bufs=1, space="PSUM"))

        W = {}
        for k in d:
            if k in ('xa0', 'xa08', 'xa016', 'xrh', 'xrl'):
                continue
            W[k] = wpool.tile(list(d[k].shape), d[k].dtype, tag=k, name=k)
            nc.sync.dma_start(W[k][:], d[k][:])

        rpool = ctx.enter_context(tc.tile_pool(name="r", bufs=2))
        res1 = apool.tile([64, SC, 32, 32], F32, tag="res1")
        bin1 = apool.tile([64, CH, 34, 34], FP8, tag="bin1")
        bin1t = apool.tile([64, CH, 34, 34], FP8, tag="bin1t")
        res2 = apool.tile([128, 1, CH, 16, 16], F32, tag="res2")
        bin2 = apool.tile([128, 1, CH, 18, 18], FP8, tag="bin2")
        bin2t = apool.tile([128, 1, CH, 18, 18], FP8, tag="bin2t")
        res3 = apool.tile([128, 2, CH, 8, 8], F32, tag="res3")
        bin3 = apool.tile([128, 2, CH, 10, 10], FP8, tag="bin3")
        bin3t = apool.tile([128, 2, CH, 10, 10], FP8, tag="bin3t")
        res4 = apool.tile([128, 4, CH, 4, 4], F32, tag="res4")
        bin4 = apool.tile([128, 4, CH, 6, 6], FP8, tag="bin4")
        bin4t = apool.tile([128, 4, CH, 6, 6], FP8, tag="bin4t")
        hsg = apool.tile([128, 4, CH, 16], BF16, tag="hsg")
        pooled = apool.tile([128, 4, B], F32, tag="pooled")

        for buf in (bin1, bin1t, bin2, bin2t, bin3, bin3t, bin4, bin4t):
            nc.vector.memset(buf[:], 0.0)

        def sgn(out_ap, in_ap, s=1.0, b=0.0):
            nc.scalar.activation(out_ap, in_ap, AT.Sign, bias=b, scale=s)

        def psum(npart, nfree):
            ps = ppool.tile([128, 512], F32, tag="ps")
            return ps[:npart, :nfree]

        for c in range(NCH):
            # ======== STEM + STAGE1 in half-chunks of SC images ========
            for sc in range(2):
                goff = c * CH + sc * SC          # global image offset in B
                loff = sc * SC                   # offset within chunk buffers
                gsl = slice(goff, goff + SC)
                Rm = rpool.tile([126, SC, 32, 38], FP16, tag="Rm")
                Rk = rpool.tile([126, SC, 32, 38], FP16, tag="Rk")
                nc.vector.memset(Rm[:], 0.0)
                nc.vector.memset(Rk[:], 0.0)
                for kh in range(7):
                    y0 = max(0, 3 - kh); y1 = min(31, 34 - kh)
                    sy0 = y0 + kh - 3
                    n_y = y1 - y0 + 1
                    for im in range(SC):
                        gim = goff + im
                        for arr, buf, ro in (('xa0', Rm, 0), ('xrh', Rk, 0),
                                             ('xrl', Rk, 21), ('xa08', Rk, 63),
                                             ('xa016', Rk, 84)):
                            nc.sync.dma_start(
                                buf[ro + 3 * kh:ro + 3 * kh + 3, im,
                                    y0:y1 + 1, 3:35],
                                d[arr][:, gim, sy0:sy0 + n_y, :])
                # duplicate blocks: a0 -> rows 42-62, xrh -> rows 105-125
                nc.sync.dma_start(Rk[42:63], Rm[0:21])
                nc.sync.dma_start(Rk[105:126], Rk[0:21])
                # Rm blocks kw=1..5: a0 shifted left by kw columns (K-merged taps)
                for b in range(1, 6):
                    nc.sync.dma_start(Rm[21 * b:21 * b + 21, :, :, 0:38 - b],
                                      Rm[0:21, :, :, b:38])
                for img in range(SC):
                    bimg = loff + img            # index into bin1 (chunk-local)
                    for yh in range(2):
                        ys = 16 * yh
                        pv = psum(64, 512).rearrange("p (y x) -> p y x", y=16)
                        nc.tensor.matmul(
                            pv[:], W['stwm6'][:, 0, :],
                            Rm[:, img, ys:ys + 16, 0:32],
                            start=True, stop=False)
                        nc.tensor.matmul(
                            pv[:], W['stwm'][:, 6, :],
                            Rm[0:21, img, ys:ys + 16, 6:38],
                            start=False, stop=False)
                        for kw in range(7):
                            nc.tensor.matmul(
                                pv[:], W['stwa'][:, kw, :],
                                Rk[:, img, ys:ys + 16, kw:kw + 32],
                                start=False, stop=False)
                        for kw in range(7):
                            nc.tensor.matmul(
                                pv[:], W['stwb'][:, kw, :],
                                Rk[0:42, img, ys:ys + 16, kw:kw + 32],
                                start=False, stop=(kw == 6))
                        nc.vector.tensor_scalar(
                            res1[:, img, ys:ys + 16, :], pv[:],
                            W['bn1s'][:, 0:1], W['bn1t'][:, 0:1], OP.mult, OP.add)
                        sgn(bin1[:, bimg, 1 + ys:17 + ys, 1:33],
                            res1[:, img, ys:ys + 16, :])

                # -------- stage 1 (64ch 32x32), two blocks --------
                def conv3_64(pv, src, bimg, yh, widx):
                    i = 0
                    for kh in range(3):
                        for kw in range(3):
                            nc.tensor.matmul(
                                pv[:], W['s1w'][:, kh * 3 + kw, widx, :],
                                src[:, bimg, kh + 16 * yh:kh + 16 * yh + 16,
                                    kw:kw + 32],
                                start=(i == 0), stop=(i == 8))
                            i += 1
                for blk in range(2):
                    c1, c2 = 2 * blk, 2 * blk + 1
                    for img in range(SC):
                        bimg = loff + img
                        for yh in range(2):
                            pv = psum(64, 512).rearrange("p (y x) -> p y x", y=16)
                            conv3_64(pv, bin1, bimg, yh, c1)
                            sgn(bin1t[:, bimg, 1 + 16 * yh:17 + 16 * yh, 1:33],
                                pv[:], W['s1bs'][:, c1:c1 + 1],
                                W['s1bt'][:, c1:c1 + 1])
                    for img in range(SC):
                        bimg = loff + img
                        for yh in range(2):
                            ys = 16 * yh
                            pv = psum(64, 512).rearrange("p (y x) -> p y x", y=16)
                            conv3_64(pv, bin1t, bimg, yh, c2)
                            h2 = tpool.tile([64, 16, 32], F32, tag="h2a")
                            nc.vector.tensor_scalar(
                                h2[:], pv[:], W['s1bs'][:, c2:c2 + 1],
                                W['s1bt'][:, c2:c2 + 1], OP.mult, OP.add)
                            rsl = res1[:, img, ys:ys + 16, :]
                            nc.vector.tensor_tensor(rsl, h2[:], rsl, OP.add)
                            sgn(bin1[:, bimg, 1 + ys:17 + ys, 1:33], rsl)

            # ======== STAGES 2..4 on the full chunk ========
            def stage(si, cin, cout, get_prev, resp, binc, binct, H):
                kin = (cin + 127) // 128
                kout = (cout + 127) // 128
                px = CH * H * H
                ntile = max(1, px // 512)
                npt = CH // ntile
                Ws, Wt = W[f's{si}bs'], W[f's{si}bt']
                nfree = npt * H * H

                def pview(npart=128):
                    return psum(npart, nfree).rearrange(
                        "p (n y x) -> p n y x", n=npt, y=H)

                def srb(buf, g, ns2, kh, kw):   # stride-1 rhs from own stage buf
                    return buf[:, g, ns2, kh:kh + H, kw:kw + H]

                for tidx in range(ntile):
                    ns2 = slice(tidx * npt, (tidx + 1) * npt)
                    for m in range(kout):
                        mp = slice(128 * m, 128 * m + 128)
                        pv = pview()
                        i = 0
                        for g in range(kin):
                            for kh in range(3):
                                for kw in range(3):
                                    nc.tensor.matmul(
                                        pv[:], W[f's{si}c1w'][:, g, kh * 3 + kw, mp],
                                        get_prev(g, ns2, kh, kw, 2),
                                        start=(i == 0), stop=(i == kin * 9 - 1))
                                    i += 1
                        sgn(binct[:, m, ns2, 1:1 + H, 1:1 + H], pv[:],
                            Ws[:, m, 0:1], Wt[:, m, 0:1])
                        pvd = pview()
                        for g in range(kin):
                            nc.tensor.matmul(
                                pvd[:], W[f's{si}dw'][:, g, mp],
                                get_prev(g, ns2, 1, 1, 2),
                                start=(g == 0), stop=(g == kin - 1))
                        nc.vector.tensor_scalar(
                            resp[:, m, ns2, :, :], pvd[:],
                            Ws[:, m, 1:2], Wt[:, m, 1:2], OP.mult, OP.add)

                for cidx in range(3):
                    srcbuf = binct if cidx in (0, 2) else binc
                    dstbuf = binc if cidx in (0, 2) else binct
                    bnc = 2 + cidx
                    for tidx in range(ntile):
                        ns2 = slice(tidx * npt, (tidx + 1) * npt)
                        for m in range(kout):
                            mp = slice(128 * m, 128 * m + 128)
                            pv = pview()
                            i = 0
                            for g in range(kout):
                                for kh in range(3):
                                    for kw in range(3):
                                        nc.tensor.matmul(
                                            pv[:],
                                            W[f's{si}w'][:, g, kh * 3 + kw, cidx, mp],
                                            srb(srcbuf, g, ns2, kh, kw),
                                            start=(i == 0),
                                            stop=(i == kout * 9 - 1))
                                        i += 1
                            if cidx in (0, 2):
                                h2 = tpool.tile([128, 512], F32, tag="h2b")
                                h2v = h2[:, :nfree].rearrange(
                                    "p (n y x) -> p n y x", n=npt, y=H)
                                nc.vector.tensor_scalar(
                                    h2v[:], pv[:], Ws[:, m, bnc:bnc + 1],
                                    Wt[:, m, bnc:bnc + 1], OP.mult, OP.add)
                                rsl = resp[:, m, ns2, :, :]
                                nc.vector.tensor_tensor(rsl, h2v[:], rsl, OP.add)
                                sgn(dstbuf[:, m, ns2, 1:1 + H, 1:1 + H], rsl)
                            else:
                                sgn(dstbuf[:, m, ns2, 1:1 + H, 1:1 + H], pv[:],
                                    Ws[:, m, bnc:bnc + 1], Wt[:, m, bnc:bnc + 1])

            def prev1(g, ns2, kh, kw, stride):   # from bin1 (4D, chunk-local)
                return bin1[:, ns2, kh:kh + 31:2, kw:kw + 31:2]

            def prev2(g, ns2, kh, kw, stride):
                return bin2[:, 0, ns2, kh:kh + 15:2, kw:kw + 15:2]

            def prev3(g, ns2, kh, kw, stride):
                return bin3[:, g, ns2, kh:kh + 7:2, kw:kw + 7:2]

            stage(2, 64, 128, prev1, res2, bin2, bin2t, 16)
            stage(3, 128, 256, prev2, res3, bin3, bin3t, 8)
            stage(4, 256, 512, prev3, res4, bin4, bin4t, 4)

            # ======== POOL ========
            for m in range(4):
                sgn(hsg[:, m, :, :],
                    res4[:, m, :, :, :].rearrange("p n y x -> p n (y x)"))
            acc = tpool.tile([128, 4, CH], F32, tag="poolacc")
            nc.vector.tensor_copy(acc[:], hsg[:, :, :, 0])
            for i in range(1, 16):
                nc.vector.tensor_tensor(acc[:], acc[:], hsg[:, :, :, i], OP.add)
            nc.scalar.mul(pooled[:, :, c * CH:(c + 1) * CH], acc[:], 1.0 / 16.0)

        # ======== FC ========
        psf = pfc.tile([10, B], F32, tag="ps_fc")
        for g in range(4):
            nc.tensor.matmul(psf[:], W['fcw'][:, g, :], pooled[:, g, :],
                             start=(g == 0), stop=(g == 3))
        osb = tpool.tile([10, B], F32, tag="osb")
        nc.vector.tensor_scalar(osb[:], psf[:], W['fcb'][:, 0:1],
                                W['fcs'][:, 0:1], OP.add, OP.mult)
        nc.sync.dma_start(out_t[:], osb[:])
        ctx.close()

    _split_waits(nc)
    return nc


_NC_CACHE = None


def _get_nc():
    global _NC_CACHE
    if _NC_CACHE is None:
        _NC_CACHE = _build()
    return _NC_CACHE


def kernel(x, params, _trace=False):
    rep, shards = _prep_inputs(x, params)
    nc = _get_nc()
    in_maps = []
    for c in range(NCORES):
        m = dict(rep)
        m.update(shards[c])
        in_maps.append(m)
    res = run_bass_kernel_spmd(nc, in_maps, core_ids=list(range(NCORES)))
    outs = [r['out'].T for r in res.results]
    full = np.ascontiguousarray(np.concatenate(outs, axis=0), dtype=np.float32)
    if _trace:
        return full, res
    return full


# revision 14
# speedup vs baseline: 1.2567x; 1.0397x over previous
"""BitwiseResnet18 forward on 8 trn2 NeuronCores — pure batch data-parallel.

Numerics (matches jax-f32 CPU reference through every sign()):
- stem conv: exact-integer main term (a0*u0 on 8-bit grids in bf16 -> exact
  f32 PSUM integer accumulation) accumulated FIRST, then an algebraically
  exact f32 correction (a0*wr + xr*w) whose roundings are relative to the
  final value.  |err| <~ 1e-7, below the minimum sign margin (~1e-6).
- binarized convs: +-1 products in fp8, integer sums in f32 PSUM: exact.
- BN affines / residual adds: f32 mul-round-add-round on DVE, same as XLA.
"""
import copy
import numpy as np
import ml_dtypes

import concourse.bass as bass
import concourse.mybir as mybir
import concourse.tile as tile
from concourse.bass_utils import run_bass_kernel_spmd

NCORES = 8
B = 64          # per-core batch
CH = 8          # images per chunk (stages 2-4); stem/stage1 use CH//2
SC = CH // 2
F32 = mybir.dt.float32
BF16 = mybir.dt.bfloat16
FP8 = mybir.dt.float8e4
NP_FP8 = mybir.dt.np(FP8)
NP_BF16 = ml_dtypes.bfloat16

STAGES = [(64, 64, 1), (64, 128, 2), (128, 256, 2), (256, 512, 2)]


# --------------------------------------------------------------- wait splitter
def _split_waits(nc, max_waits=1):
    """The walrus build here accepts only one sync-wait per instruction;
    move extra waits onto injected EventSemaphore carriers."""
    mod = nc.m
    counter = [0]

    def carrier(engine, waits, debug):
        counter[0] += 1
        si = mybir.SyncInfo(on_wait=list(waits), on_update=[])
        return mybir.InstEventSemaphore(
            name=f"WSPLIT-{counter[0]}", engine=engine, sync_info=si,
            ins=[], outs=[], debug=debug)

    new_functions = []
    for function in mod.functions:
        nf = copy.replace(function, blocks=[])
        nf.set_allocations_from_list(function.allocations)
        for block in function.blocks:
            insts = []
            for inst in block.instructions:
                si = inst.sync_info
                waits = list(si.on_wait) if si is not None and si.on_wait else []
                if len(waits) > max_waits:
                    head, keep = waits[:-max_waits], waits[-max_waits:]
                    for i in range(0, len(head), max_waits):
                        insts.append(carrier(inst.engine, head[i:i + max_waits],
                                             inst.debug))
                    inst = copy.replace(inst, sync_info=mybir.SyncInfo(
                        on_wait=keep,
                        on_update=list(si.on_update) if si.on_update else []))
                insts.append(inst)
            nf.blocks.append(copy.replace(block, instructions=insts))
        new_functions.append(nf)
    new_mod = copy.replace(mod, functions=[])
    for f in new_functions:
        new_mod.functions.append(f)
    nc.m = new_mod
    return nc


# ------------------------------------------------------------------ host prep
def _f32(a):
    return np.asarray(a, dtype=np.float32)


def _bn_fold(p):
    var = _f32(p['var']); gamma = _f32(p['gamma'])
    beta = _f32(p['beta']); mean = _f32(p['mean'])
    inv = (np.float32(1.0) / np.sqrt(var + np.float32(1e-5))).astype(np.float32)
    s = (gamma * inv).astype(np.float32)
    t = (beta - (mean * gamma) * inv).astype(np.float32)
    return s, t


def _binw(w):
    w = np.asarray(w, dtype=np.float32)
    sw = np.where(w >= 0, np.float32(1), np.float32(-1))
    lhsT = np.transpose(sw, (1, 2, 3, 0)).reshape(sw.shape[1], 9, sw.shape[0])
    return np.ascontiguousarray(lhsT).astype(NP_FP8)          # [Cin, 9, Cout]


def _binw1x1(w):
    w = np.asarray(w, dtype=np.float32)[:, :, 0, 0]
    sw = np.where(w >= 0, np.float32(1), np.float32(-1))
    return np.ascontiguousarray(sw.T).astype(NP_FP8)          # [Cin, Cout]


def _prep_inputs(x, params):
    x = np.asarray(x, dtype=np.float32)
    x64 = x.astype(np.float64)
    a0 = np.round(x64 * 32.0) / 32.0          # 2^-5 grid, ints <= 173
    resid = x64 - a0
    xrh_s = (resid * 2.0 ** 12).astype(np.float16)       # scaled limb
    xrh = xrh_s.astype(np.float64) * 2.0 ** -12
    xrl_s = ((resid - xrh) * 2.0 ** 24).astype(np.float16)

    w64 = np.asarray(params['conv1_w'], np.float32).astype(np.float64)
    u0 = np.round(w64 * 256.0) / 256.0        # 2^-8 grid, ints <= 167 (bf16 ok)
    wr = w64 - u0
    wr1 = np.round(wr * 2.0 ** 16) * 2.0 ** -16
    wr2 = np.round((wr - wr1) * 2.0 ** 24) * 2.0 ** -24
    wr3 = np.round((wr - wr1 - wr2) * 2.0 ** 32) * 2.0 ** -32
    um = np.zeros((21, 7, 64), np.float64)    # [row, kw, Cout]
    wa = np.zeros((126, 7, 64), np.float64)
    wb = np.zeros((42, 7, 64), np.float64)
    for kw in range(7):
        for kh in range(7):
            for ci in range(3):
                r = kh * 3 + ci
                um[r, kw, :] = u0[:, ci, kh, kw]
                wa[r, kw, :] = u0[:, ci, kh, kw] * 2.0 ** -12      # x xrh_s
                wa[21 + r, kw, :] = u0[:, ci, kh, kw] * 2.0 ** -24  # x xrl_s
                wa[42 + r, kw, :] = wr1[:, ci, kh, kw]              # x a0
                wa[63 + r, kw, :] = wr2[:, ci, kh, kw] * 2.0 ** 8   # x a0*2^-8
                wa[84 + r, kw, :] = wr3[:, ci, kh, kw] * 2.0 ** 16  # x a0*2^-16
                wa[105 + r, kw, :] = wr1[:, ci, kh, kw] * 2.0 ** -12  # x xrh_s
                wb[r, kw, :] = wr2[:, ci, kh, kw] * 2.0 ** -12      # x xrh_s
                wb[21 + r, kw, :] = wr1[:, ci, kh, kw] * 2.0 ** -24  # x xrl_s
    um6 = np.concatenate([um[:, kw, :] for kw in range(6)], axis=0)  # [126, 64]
    wb3 = np.zeros((126, 2, 64), np.float64)
    for g in range(2):
        for b in range(3):
            wb3[42 * b:42 * b + 42, g, :] = wb[:, 3 * g + b, :]
    rep = {'stwm': um.astype(NP_BF16), 'stwm6': um6.reshape(126, 1, 64).astype(NP_BF16),
           'stwa': wa.astype(NP_BF16), 'stwb': wb.astype(NP_BF16),
           'stwb3': wb3.astype(NP_BF16)}

    s, t = _bn_fold(params['bn1'])
    rep['bn1s'] = s.reshape(64, 1)
    rep['bn1t'] = t.reshape(64, 1)

    st = params['stages']
    w1 = np.stack([_binw(st[0][0]['conv1_w']), _binw(st[0][0]['conv2_w']),
                   _binw(st[0][1]['conv1_w']), _binw(st[0][1]['conv2_w'])], axis=2)
    rep['s1w'] = np.ascontiguousarray(w1)                     # [64, 9, 4, 64]
    bs = [_bn_fold(st[0][0]['bn1']), _bn_fold(st[0][0]['bn2']),
          _bn_fold(st[0][1]['bn1']), _bn_fold(st[0][1]['bn2'])]
    rep['s1bs'] = np.ascontiguousarray(np.stack([b[0] for b in bs], 1))
    rep['s1bt'] = np.ascontiguousarray(np.stack([b[1] for b in bs], 1))

    for si, (cin, cout, stride) in enumerate(STAGES[1:], start=2):
        b1, b2 = st[si - 1]
        kin = (cin + 127) // 128
        kout = (cout + 127) // 128
        c1 = _binw(b1['conv1_w'])
        dw = _binw1x1(b1['down_w'])
        rest = np.stack([_binw(b1['conv2_w']), _binw(b2['conv1_w']),
                         _binw(b2['conv2_w'])], axis=2)       # [cout,9,3,cout]
        kp = min(cin, 128)
        rep[f's{si}c1w'] = np.ascontiguousarray(
            c1.reshape(kin, kp, 9, cout).transpose(1, 0, 2, 3))
        rep[f's{si}dw'] = np.ascontiguousarray(
            dw.reshape(kin, kp, cout).transpose(1, 0, 2))
        rep[f's{si}w'] = np.ascontiguousarray(
            rest.reshape(kout, 128, 9, 3, cout).transpose(1, 0, 2, 3, 4))
        bs = [_bn_fold(b1['bn1']), _bn_fold(b1['down_bn']), _bn_fold(b1['bn2']),
              _bn_fold(b2['bn1']), _bn_fold(b2['bn2'])]
        rep[f's{si}bs'] = np.ascontiguousarray(
            np.stack([b[0] for b in bs], 1).reshape(kout, 128, 5).transpose(1, 0, 2))
        rep[f's{si}bt'] = np.ascontiguousarray(
            np.stack([b[1] for b in bs], 1).reshape(kout, 128, 5).transpose(1, 0, 2))

    fcw = _f32(params['fc_w'])
    rep['fcw'] = np.ascontiguousarray(fcw.T.reshape(4, 128, 10).transpose(1, 0, 2))
    rep['fcb'] = _f32(params['fc_b']).reshape(10, 1)
    rep['fcs'] = _f32(params['scale']).reshape(10, 1)

    shards = []
    a0h = a0.astype(np.float16)
    a08 = (a0 * 2.0 ** -8).astype(np.float16)
    a016 = (a0 * 2.0 ** -16).astype(np.float16)
    for c in range(NCORES):
        sl = slice(c * B, (c + 1) * B)
        shards.append({
            'xa0': np.ascontiguousarray(a0h[sl].transpose(1, 0, 2, 3)),
            'xa08': np.ascontiguousarray(a08[sl].transpose(1, 0, 2, 3)),
            'xa016': np.ascontiguousarray(a016[sl].transpose(1, 0, 2, 3)),
            'xrh': np.ascontiguousarray(xrh_s[sl].transpose(1, 0, 2, 3)),
            'xrl': np.ascontiguousarray(xrl_s[sl].transpose(1, 0, 2, 3)),
        })
    return rep, shards


# ------------------------------------------------------------------ bass build
def _build():
    nc = bass.Bass()
    NCH = B // CH
    AT = mybir.ActivationFunctionType
    OP = mybir.AluOpType

    d = {}
    FP16 = mybir.dt.float16
    for nm in ('xa0', 'xa08', 'xa016', 'xrh', 'xrl'):
        d[nm] = nc.dram_tensor(nm, [3, B, 32, 32], FP16, kind="ExternalInput")
    d['stwm'] = nc.dram_tensor("stwm", [21, 7, 64], BF16, kind="ExternalInput")
    d['stwm6'] = nc.dram_tensor("stwm6", [126, 1, 64], BF16, kind="ExternalInput")
    d['stwb3'] = nc.dram_tensor("stwb3", [126, 2, 64], BF16, kind="ExternalInput")
    d['stwa'] = nc.dram_tensor("stwa", [126, 7, 64], BF16, kind="ExternalInput")
    d['stwb'] = nc.dram_tensor("stwb", [42, 7, 64], BF16, kind="ExternalInput")
    d['bn1s'] = nc.dram_tensor("bn1s", [64, 1], F32, kind="ExternalInput")
    d['bn1t'] = nc.dram_tensor("bn1t", [64, 1], F32, kind="ExternalInput")
    d['s1w'] = nc.dram_tensor("s1w", [64, 9, 4, 64], FP8, kind="ExternalInput")
    d['s1bs'] = nc.dram_tensor("s1bs", [64, 4], F32, kind="ExternalInput")
    d['s1bt'] = nc.dram_tensor("s1bt", [64, 4], F32, kind="ExternalInput")
    for si, (cin, cout, stride) in enumerate(STAGES[1:], start=2):
        kin = (cin + 127) // 128
        kout = (cout + 127) // 128
        kp = min(cin, 128)
        d[f's{si}c1w'] = nc.dram_tensor(f"s{si}c1w", [kp, kin, 9, cout], FP8, kind="ExternalInput")
        d[f's{si}dw'] = nc.dram_tensor(f"s{si}dw", [kp, kin, cout], FP8, kind="ExternalInput")
        d[f's{si}w'] = nc.dram_tensor(f"s{si}w", [128, kout, 9, 3, cout], FP8, kind="ExternalInput")
        d[f's{si}bs'] = nc.dram_tensor(f"s{si}bs", [128, kout, 5], F32, kind="ExternalInput")
        d[f's{si}bt'] = nc.dram_tensor(f"s{si}bt", [128, kout, 5], F32, kind="ExternalInput")
    d['fcw'] = nc.dram_tensor("fcw", [128, 4, 10], F32, kind="ExternalInput")
    d['fcb'] = nc.dram_tensor("fcb", [10, 1], F32, kind="ExternalInput")
    d['fcs'] = nc.dram_tensor("fcs", [10, 1], F32, kind="ExternalInput")
    out_t = nc.dram_tensor("out", [10, B], F32, kind="ExternalOutput")

    with tile.TileContext(nc) as tc:
        import contextlib
        ctx = contextlib.ExitStack()
        wpool = ctx.enter_context(tc.tile_pool(name="w", bufs=1))
        apool = ctx.enter_context(tc.tile_pool(name="a", bufs=1))
        tpool = ctx.enter_context(tc.tile_pool(name="t", bufs=2))
        ppool = ctx.enter_context(tc.tile_pool(name="p", bufs=6, space="PSUM"))
        pfc = ctx.enter_context(tc.tile_pool(name="pfc", bufs=1, space="PSUM"))

        W = {}
        for k in d:
            if k in ('xa0', 'xa08', 'xa016', 'xrh', 'xrl'):
                continue
            W[k] = wpool.tile(list(d[k].shape), d[k].dtype, tag=k, name=k)
            nc.sync.dma_start(W[k][:], d[k][:])

        rpool = ctx.enter_context(tc.tile_pool(name="r", bufs=2))
        res1 = apool.tile([64, SC, 32, 32], F32, tag="res1")
        bin1 = apool.tile([64, CH, 34, 34], FP8, tag="bin1")
        bin1t = apool.tile([64, CH, 34, 34], FP8, tag="bin1t")
        res2 = apool.tile([128, 1, CH, 16, 16], F32, tag="res2")
        bin2 = apool.tile([128, 1, CH, 18, 18], FP8, tag="bin2")
        bin2t = apool.tile([128, 1, CH, 18, 18], FP8, tag="bin2t")
        res3 = apool.tile([128, 2, CH, 8, 8], F32, tag="res3")
        bin3 = apool.tile([128, 2, CH, 10, 10], FP8, tag="bin3")
        bin3t = apool.tile([128, 2, CH, 10, 10], FP8, tag="bin3t")
        res4 = apool.tile([128, 4, CH, 4, 4], F32, tag="res4")
        bin4 = apool.tile([128, 4, CH, 6, 6], FP8, tag="bin4")
        bin4t = apool.tile([128, 4, CH, 6, 6], FP8, tag="bin4t")
        hsg = apool.tile([128, 4, CH, 16], BF16, tag="hsg")
        RkB = apool.tile([126, SC, 32, 38], FP16, tag="RkB")
        nc.vector.memset(RkB[:], 0.0)
        pooled = apool.tile([128, 4, B], F32, tag="pooled")

        for buf in (bin1, bin1t, bin2, bin2t, bin3, bin3t, bin4, bin4t):
            nc.vector.memset(buf[:], 0.0)

        def sgn(out_ap, in_ap, s=1.0, b=0.0):
            nc.scalar.activation(out_ap, in_ap, AT.Sign, bias=b, scale=s)

        def psum(npart, nfree):
            ps = ppool.tile([128, 512], F32, tag="ps")
            return ps[:npart, :nfree]

        for c in range(NCH):
            # ======== STEM + STAGE1 in half-chunks of SC images ========
            for sc in range(2):
                goff = c * CH + sc * SC          # global image offset in B
                loff = sc * SC                   # offset within chunk buffers
                gsl = slice(goff, goff + SC)
                Rm = rpool.tile([126, SC, 32, 38], FP16, tag="Rm")
                Rk = rpool.tile([126, SC, 32, 38], FP16, tag="Rk")
                nc.vector.memset(Rm[:], 0.0)
                nc.vector.memset(Rk[:], 0.0)
                for kh in range(7):
                    y0 = max(0, 3 - kh); y1 = min(31, 34 - kh)
                    sy0 = y0 + kh - 3
                    n_y = y1 - y0 + 1
                    for im in range(SC):
                        gim = goff + im
                        for arr, buf, ro in (('xa0', Rm, 0), ('xrh', Rk, 0),
                                             ('xrl', Rk, 21), ('xa08', Rk, 63),
                                             ('xa016', Rk, 84)):
                            nc.sync.dma_start(
                                buf[ro + 3 * kh:ro + 3 * kh + 3, im,
                                    y0:y1 + 1, 3:35],
                                d[arr][:, gim, sy0:sy0 + n_y, :])
                # duplicate blocks: a0 -> rows 42-62, xrh -> rows 105-125
                nc.sync.dma_start(Rk[42:63], Rm[0:21])
                nc.sync.dma_start(Rk[105:126], Rk[0:21])
                # Rm blocks kw=1..5: a0 shifted left by kw columns (K-merged taps)
                for b in range(1, 6):
                    nc.sync.dma_start(Rm[21 * b:21 * b + 21, :, :, 0:38 - b],
                                      Rm[0:21, :, :, b:38])
                # RkB: [xrh;xrl] shifted by 0/1/2 columns for corrB kw-triples
                nc.sync.dma_start(RkB[0:42], Rk[0:42])
                for b in range(1, 3):
                    nc.sync.dma_start(RkB[42 * b:42 * b + 42, :, :, 0:38 - b],
                                      Rk[0:42, :, :, b:38])
                for img in range(SC):
                    bimg = loff + img            # index into bin1 (chunk-local)
                    for yh in range(2):
                        ys = 16 * yh
                        pv = psum(64, 512).rearrange("p (y x) -> p y x", y=16)
                        nc.tensor.matmul(
                            pv[:], W['stwm6'][:, 0, :],
                            Rm[:, img, ys:ys + 16, 0:32],
                            start=True, stop=False)
                        nc.tensor.matmul(
                            pv[:], W['stwm'][:, 6, :],
                            Rm[0:21, img, ys:ys + 16, 6:38],
                            start=False, stop=False)
                        for kw in range(7):
                            nc.tensor.matmul(
                                pv[:], W['stwa'][:, kw, :],
                                Rk[:, img, ys:ys + 16, kw:kw + 32],
                                start=False, stop=False)
                        for g in range(2):
                            nc.tensor.matmul(
                                pv[:], W['stwb3'][:, g, :],
                                RkB[:, img, ys:ys + 16, 3 * g:3 * g + 32],
                                start=False, stop=False)
                        nc.tensor.matmul(
                            pv[:], W['stwb'][:, 6, :],
                            Rk[0:42, img, ys:ys + 16, 6:38],
                            start=False, stop=True)
                        nc.vector.tensor_scalar(
                            res1[:, img, ys:ys + 16, :], pv[:],
                            W['bn1s'][:, 0:1], W['bn1t'][:, 0:1], OP.mult, OP.add)
                        sgn(bin1[:, bimg, 1 + ys:17 + ys, 1:33],
                            res1[:, img, ys:ys + 16, :])

                # -------- stage 1 (64ch 32x32), two blocks --------
                def conv3_64(pv, src, bimg, yh, widx):
                    i = 0
                    for kh in range(3):
                        for kw in range(3):
                            nc.tensor.matmul(
                                pv[:], W['s1w'][:, kh * 3 + kw, widx, :],
                                src[:, bimg, kh + 16 * yh:kh + 16 * yh + 16,
                                    kw:kw + 32],
                                start=(i == 0), stop=(i == 8))
                            i += 1
                for blk in range(2):
                    c1, c2 = 2 * blk, 2 * blk + 1
                    for img in range(SC):
                        bimg = loff + img
                        for yh in range(2):
                            pv = psum(64, 512).rearrange("p (y x) -> p y x", y=16)
                            conv3_64(pv, bin1, bimg, yh, c1)
                            sgn(bin1t[:, bimg, 1 + 16 * yh:17 + 16 * yh, 1:33],
                                pv[:], W['s1bs'][:, c1:c1 + 1],
                                W['s1bt'][:, c1:c1 + 1])
                    for img in range(SC):
                        bimg = loff + img
                        for yh in range(2):
                            ys = 16 * yh
                            pv = psum(64, 512).rearrange("p (y x) -> p y x", y=16)
                            conv3_64(pv, bin1t, bimg, yh, c2)
                            h2 = tpool.tile([64, 16, 32], F32, tag="h2a")
                            nc.vector.tensor_scalar(
                                h2[:], pv[:], W['s1bs'][:, c2:c2 + 1],
                                W['s1bt'][:, c2:c2 + 1], OP.mult, OP.add)
                            rsl = res1[:, img, ys:ys + 16, :]
                            nc.vector.tensor_tensor(rsl, h2[:], rsl, OP.add)
                            sgn(bin1[:, bimg, 1 + ys:17 + ys, 1:33], rsl)

            # ======== STAGES 2..4 on the full chunk ========
            def stage(si, cin, cout, get_prev, resp, binc, binct, H):
                kin = (cin + 127) // 128
                kout = (cout + 127) // 128
                px = CH * H * H
                ntile = max(1, px // 512)
                npt = CH // ntile
                Ws, Wt = W[f's{si}bs'], W[f's{si}bt']
                nfree = npt * H * H

                def pview(npart=128):
                    return psum(npart, nfree).rearrange(
                        "p (n y x) -> p n y x", n=npt, y=H)

                def srb(buf, g, ns2, kh, kw):   # stride-1 rhs from own stage buf
                    return buf[:, g, ns2, kh:kh + H, kw:kw + H]

                for tidx in range(ntile):
                    ns2 = slice(tidx * npt, (tidx + 1) * npt)
                    for m in range(kout):
                        mp = slice(128 * m, 128 * m + 128)
                        pv = pview()
                        i = 0
                        for g in range(kin):
                            for kh in range(3):
                                for kw in range(3):
                                    nc.tensor.matmul(
                                        pv[:], W[f's{si}c1w'][:, g, kh * 3 + kw, mp],
                                        get_prev(g, ns2, kh, kw, 2),
                                        start=(i == 0), stop=(i == kin * 9 - 1))
                                    i += 1
                        sgn(binct[:, m, ns2, 1:1 + H, 1:1 + H], pv[:],
                            Ws[:, m, 0:1], Wt[:, m, 0:1])
                        pvd = pview()
                        for g in range(kin):
                            nc.tensor.matmul(
                                pvd[:], W[f's{si}dw'][:, g, mp],
                                get_prev(g, ns2, 1, 1, 2),
                                start=(g == 0), stop=(g == kin - 1))
                        nc.vector.tensor_scalar(
                            resp[:, m, ns2, :, :], pvd[:],
                            Ws[:, m, 1:2], Wt[:, m, 1:2], OP.mult, OP.add)

                for cidx in range(3):
                    srcbuf = binct if cidx in (0, 2) else binc
                    dstbuf = binc if cidx in (0, 2) else binct
                    bnc = 2 + cidx
                    for tidx in range(ntile):
                        ns2 = slice(tidx * npt, (tidx + 1) * npt)
                        for m in range(kout):
                            mp = slice(128 * m, 128 * m + 128)
                            pv = pview()
                            i = 0
                            for g in range(kout):
                                for kh in range(3):
                                    for kw in range(3):
                                        nc.tensor.matmul(
                                            pv[:],
                                            W[f's{si}w'][:, g, kh * 3 + kw, cidx, mp],
                                            srb(srcbuf, g, ns2, kh, kw),
                                            start=(i == 0),
                                            stop=(i == kout * 9 - 1))
                                        i += 1
                            if cidx in (0, 2):
                                h2 = tpool.tile([128, 512], F32, tag="h2b")
                                h2v = h2[:, :nfree].rearrange(
                                    "p (n y x) -> p n y x", n=npt, y=H)
                                nc.vector.tensor_scalar(
                                    h2v[:], pv[:], Ws[:, m, bnc:bnc + 1],
                                    Wt[:, m, bnc:bnc + 1], OP.mult, OP.add)
                                rsl = resp[:, m, ns2, :, :]
                                nc.vector.tensor_tensor(rsl, h2v[:], rsl, OP.add)
                                sgn(dstbuf[:, m, ns2, 1:1 + H, 1:1 + H], rsl)
                            else:
                                sgn(dstbuf[:, m, ns2, 1:1 + H, 1:1 + H], pv[:],
                                    Ws[:, m, bnc:bnc + 1], Wt[:, m, bnc:bnc + 1])

            def prev1(g, ns2, kh, kw, stride):   # from bin1 (4D, chunk-local)
                return bin1[:, ns2, kh:kh + 31:2, kw:kw + 31:2]

            def prev2(g, ns2, kh, kw, stride):
                return bin2[:, 0, ns2, kh:kh + 15:2, kw:kw + 15:2]

            def prev3(g, ns2, kh, kw, stride):
                return bin3[:, g, ns2, kh:kh + 7:2, kw:kw + 7:2]

            stage(2, 64, 128, prev1, res2, bin2, bin2t, 16)
            stage(3, 128, 256, prev2, res3, bin3, bin3t, 8)
            stage(4, 256, 512, prev3, res4, bin4, bin4t, 4)

            # ======== POOL ========
            for m in range(4):
                sgn(hsg[:, m, :, :],
                    res4[:, m, :, :, :].rearrange("p n y x -> p n (y x)"))
            acc = tpool.tile([128, 4, CH], F32, tag="poolacc")
            nc.vector.tensor_copy(acc[:], hsg[:, :, :, 0])
            for i in range(1, 16):
                nc.vector.tensor_tensor(acc[:], acc[:], hsg[:, :, :, i], OP.add)
            nc.scalar.mul(pooled[:, :, c * CH:(c + 1) * CH], acc[:], 1.0 / 16.0)

        # ======== FC ========
        psf = pfc.tile([10, B], F32, tag="ps_fc")
        for g in range(4):
            nc.tensor.matmul(psf[:], W['fcw'][:, g, :], pooled[:, g, :],
                             start=(g == 0), stop=(g == 3))
        osb = tpool.tile([10, B], F32, tag="osb")
        nc.vector.tensor_scalar(osb[:], psf[:], W['fcb'][:, 0:1],
                                W['fcs'][:, 0:1], OP.add, OP.mult)
        nc.sync.dma_start(out_t[:], osb[:])
        ctx.close()

    _split_waits(nc)
    return nc


_NC_CACHE = None


def _get_nc():
    global _NC_CACHE
    if _NC_CACHE is None:
        _NC_CACHE = _build()
    return _NC_CACHE


def kernel(x, params, _trace=False):
    rep, shards = _prep_inputs(x, params)
    nc = _get_nc()
    in_maps = []
    for c in range(NCORES):
        m = dict(rep)
        m.update(shards[c])
        in_maps.append(m)
    res = run_bass_kernel_spmd(nc, in_maps, core_ids=list(range(NCORES)))
    outs = [r['out'].T for r in res.results]
    full = np.ascontiguousarray(np.concatenate(outs, axis=0), dtype=np.float32)
    if _trace:
        return full, res
    return full


# revision 18
# speedup vs baseline: 1.4600x; 1.1618x over previous
"""BitwiseResnet18 forward on 8 trn2 NeuronCores — pure batch data-parallel.

Numerics (matches jax-f32 CPU reference through every sign()):
- stem conv: exact-integer main term (a0*u0 on 8-bit grids in bf16 -> exact
  f32 PSUM integer accumulation) accumulated FIRST, then an algebraically
  exact f32 correction (a0*wr + xr*w) whose roundings are relative to the
  final value.  |err| <~ 1e-7, below the minimum sign margin (~1e-6).
- binarized convs: +-1 products in fp8, integer sums in f32 PSUM: exact.
- BN affines / residual adds: f32 mul-round-add-round on DVE, same as XLA.
"""
import copy
import numpy as np
import ml_dtypes

import concourse.bass as bass
import concourse.mybir as mybir
import concourse.tile as tile
from concourse.bass_utils import run_bass_kernel_spmd

NCORES = 8
B = 64          # per-core batch
CH = 8          # images per chunk (stages 2-4); stem/stage1 use CH//2
SC = CH // 2
F32 = mybir.dt.float32
BF16 = mybir.dt.bfloat16
FP8 = mybir.dt.float8e4
NP_FP8 = mybir.dt.np(FP8)
NP_BF16 = ml_dtypes.bfloat16

STAGES = [(64, 64, 1), (64, 128, 2), (128, 256, 2), (256, 512, 2)]


# --------------------------------------------------------------- wait splitter
def _split_waits(nc, max_waits=1):
    """The walrus build here accepts only one sync-wait per instruction;
    move extra waits onto injected EventSemaphore carriers."""
    mod = nc.m
    counter = [0]

    def carrier(engine, waits, debug):
        counter[0] += 1
        si = mybir.SyncInfo(on_wait=list(waits), on_update=[])
        return mybir.InstEventSemaphore(
            name=f"WSPLIT-{counter[0]}", engine=engine, sync_info=si,
            ins=[], outs=[], debug=debug)

    new_functions = []
    for function in mod.functions:
        nf = copy.replace(function, blocks=[])
        nf.set_allocations_from_list(function.allocations)
        for block in function.blocks:
            insts = []
            for inst in block.instructions:
                si = inst.sync_info
                waits = list(si.on_wait) if si is not None and si.on_wait else []
                if len(waits) > max_waits:
                    head, keep = waits[:-max_waits], waits[-max_waits:]
                    for i in range(0, len(head), max_waits):
                        insts.append(carrier(inst.engine, head[i:i + max_waits],
                                             inst.debug))
                    inst = copy.replace(inst, sync_info=mybir.SyncInfo(
                        on_wait=keep,
                        on_update=list(si.on_update) if si.on_update else []))
                insts.append(inst)
            nf.blocks.append(copy.replace(block, instructions=insts))
        new_functions.append(nf)
    new_mod = copy.replace(mod, functions=[])
    for f in new_functions:
        new_mod.functions.append(f)
    nc.m = new_mod
    return nc


# ------------------------------------------------------------------ host prep
def _f32(a):
    return np.asarray(a, dtype=np.float32)


def _bn_fold(p):
    var = _f32(p['var']); gamma = _f32(p['gamma'])
    beta = _f32(p['beta']); mean = _f32(p['mean'])
    inv = (np.float32(1.0) / np.sqrt(var + np.float32(1e-5))).astype(np.float32)
    s = (gamma * inv).astype(np.float32)
    t = (beta - (mean * gamma) * inv).astype(np.float32)
    return s, t


def _binw(w):
    w = np.asarray(w, dtype=np.float32)
    sw = np.where(w >= 0, np.float32(1), np.float32(-1))
    lhsT = np.transpose(sw, (1, 2, 3, 0)).reshape(sw.shape[1], 9, sw.shape[0])
    return np.ascontiguousarray(lhsT).astype(NP_FP8)          # [Cin, 9, Cout]


def _binw1x1(w):
    w = np.asarray(w, dtype=np.float32)[:, :, 0, 0]
    sw = np.where(w >= 0, np.float32(1), np.float32(-1))
    return np.ascontiguousarray(sw.T).astype(NP_FP8)          # [Cin, Cout]


def _prep_inputs(x, params):
    x = np.asarray(x, dtype=np.float32)
    x64 = x.astype(np.float64)
    a0 = np.round(x64 * 32.0) / 32.0          # 2^-5 grid, ints <= 173
    resid = x64 - a0
    xrh_s = (resid * 2.0 ** 12).astype(np.float16)       # scaled limb
    xrh = xrh_s.astype(np.float64) * 2.0 ** -12
    xrl_s = ((resid - xrh) * 2.0 ** 24).astype(np.float16)

    w64 = np.asarray(params['conv1_w'], np.float32).astype(np.float64)
    u0 = np.round(w64 * 256.0) / 256.0        # 2^-8 grid, ints <= 167 (bf16 ok)
    wr = w64 - u0
    wr1 = np.round(wr * 2.0 ** 16) * 2.0 ** -16
    wr2 = np.round((wr - wr1) * 2.0 ** 24) * 2.0 ** -24
    wr3 = np.round((wr - wr1 - wr2) * 2.0 ** 32) * 2.0 ** -32
    um = np.zeros((21, 7, 64), np.float64)    # [row, kw, Cout]
    wa = np.zeros((126, 7, 64), np.float64)
    wb = np.zeros((42, 7, 64), np.float64)
    for kw in range(7):
        for kh in range(7):
            for ci in range(3):
                r = kh * 3 + ci
                um[r, kw, :] = u0[:, ci, kh, kw]
                wa[r, kw, :] = u0[:, ci, kh, kw] * 2.0 ** -12      # x xrh_s
                wa[21 + r, kw, :] = u0[:, ci, kh, kw] * 2.0 ** -24  # x xrl_s
                wa[42 + r, kw, :] = wr1[:, ci, kh, kw]              # x a0
                wa[63 + r, kw, :] = wr2[:, ci, kh, kw] * 2.0 ** 8   # x a0*2^-8
                wa[84 + r, kw, :] = wr3[:, ci, kh, kw] * 2.0 ** 16  # x a0*2^-16
                wa[105 + r, kw, :] = wr1[:, ci, kh, kw] * 2.0 ** -12  # x xrh_s
                wb[r, kw, :] = wr2[:, ci, kh, kw] * 2.0 ** -12      # x xrh_s
                wb[21 + r, kw, :] = wr1[:, ci, kh, kw] * 2.0 ** -24  # x xrl_s
    um6 = np.concatenate([um[:, kw, :] for kw in range(6)], axis=0)  # [126, 64]
    wb3 = np.zeros((126, 2, 64), np.float64)
    for g in range(2):
        for b in range(3):
            wb3[42 * b:42 * b + 42, g, :] = wb[:, 3 * g + b, :]
    rep = {'stwm': um.astype(NP_BF16), 'stwm6': um6.reshape(126, 1, 64).astype(NP_BF16),
           'stwa': wa.astype(NP_BF16), 'stwb': wb.astype(NP_BF16),
           'stwb3': wb3.astype(NP_BF16)}

    s, t = _bn_fold(params['bn1'])
    rep['bn1s'] = s.reshape(64, 1)
    rep['bn1t'] = t.reshape(64, 1)

    st = params['stages']
    w1 = np.stack([_binw(st[0][0]['conv1_w']), _binw(st[0][0]['conv2_w']),
                   _binw(st[0][1]['conv1_w']), _binw(st[0][1]['conv2_w'])], axis=2)
    rep['s1w'] = np.ascontiguousarray(w1[:, 2::3])            # [64, 3, 4, 64] kw=2
    w1m = np.zeros((128, 3, 4, 64), w1.dtype)
    for kh in range(3):
        w1m[0:64, kh] = w1[:, kh * 3 + 0]
        w1m[64:128, kh] = w1[:, kh * 3 + 1]
    rep['s1wm'] = np.ascontiguousarray(w1m)
    bs = [_bn_fold(st[0][0]['bn1']), _bn_fold(st[0][0]['bn2']),
          _bn_fold(st[0][1]['bn1']), _bn_fold(st[0][1]['bn2'])]
    rep['s1bs'] = np.ascontiguousarray(np.stack([b[0] for b in bs], 1))
    rep['s1bt'] = np.ascontiguousarray(np.stack([b[1] for b in bs], 1))

    for si, (cin, cout, stride) in enumerate(STAGES[1:], start=2):
        b1, b2 = st[si - 1]
        kin = (cin + 127) // 128
        kout = (cout + 127) // 128
        c1 = _binw(b1['conv1_w'])
        dw = _binw1x1(b1['down_w'])
        rest = np.stack([_binw(b1['conv2_w']), _binw(b2['conv1_w']),
                         _binw(b2['conv2_w'])], axis=2)       # [cout,9,3,cout]
        kp = min(cin, 128)
        rep[f's{si}c1w'] = np.ascontiguousarray(
            c1.reshape(kin, kp, 9, cout).transpose(1, 0, 2, 3))
        rep[f's{si}dw'] = np.ascontiguousarray(
            dw.reshape(kin, kp, cout).transpose(1, 0, 2))
        rep[f's{si}w'] = np.ascontiguousarray(
            rest.reshape(kout, 128, 9, 3, cout).transpose(1, 0, 2, 3, 4))
        bs = [_bn_fold(b1['bn1']), _bn_fold(b1['down_bn']), _bn_fold(b1['bn2']),
              _bn_fold(b2['bn1']), _bn_fold(b2['bn2'])]
        rep[f's{si}bs'] = np.ascontiguousarray(
            np.stack([b[0] for b in bs], 1).reshape(kout, 128, 5).transpose(1, 0, 2))
        rep[f's{si}bt'] = np.ascontiguousarray(
            np.stack([b[1] for b in bs], 1).reshape(kout, 128, 5).transpose(1, 0, 2))

    fcw = _f32(params['fc_w'])
    rep['fcw'] = np.ascontiguousarray(fcw.T.reshape(4, 128, 10).transpose(1, 0, 2))
    rep['fcb'] = _f32(params['fc_b']).reshape(10, 1)
    rep['fcs'] = _f32(params['scale']).reshape(10, 1)

    shards = []
    a0h = a0.astype(np.float16)
    a08 = (a0 * 2.0 ** -8).astype(np.float16)
    a016 = (a0 * 2.0 ** -16).astype(np.float16)
    for c in range(NCORES):
        sl = slice(c * B, (c + 1) * B)
        shards.append({
            'xa0': np.ascontiguousarray(a0h[sl].transpose(1, 0, 2, 3)),
            'xa08': np.ascontiguousarray(a08[sl].transpose(1, 0, 2, 3)),
            'xa016': np.ascontiguousarray(a016[sl].transpose(1, 0, 2, 3)),
            'xrh': np.ascontiguousarray(xrh_s[sl].transpose(1, 0, 2, 3)),
            'xrl': np.ascontiguousarray(xrl_s[sl].transpose(1, 0, 2, 3)),
        })
    return rep, shards


# ------------------------------------------------------------------ bass build
def _build():
    nc = bass.Bass()
    NCH = B // CH
    AT = mybir.ActivationFunctionType
    OP = mybir.AluOpType

    d = {}
    FP16 = mybir.dt.float16
    for nm in ('xa0', 'xa08', 'xa016', 'xrh', 'xrl'):
        d[nm] = nc.dram_tensor(nm, [3, B, 32, 32], FP16, kind="ExternalInput")
    d['stwm'] = nc.dram_tensor("stwm", [21, 7, 64], BF16, kind="ExternalInput")
    d['stwm6'] = nc.dram_tensor("stwm6", [126, 1, 64], BF16, kind="ExternalInput")
    d['stwb3'] = nc.dram_tensor("stwb3", [126, 2, 64], BF16, kind="ExternalInput")
    d['stwa'] = nc.dram_tensor("stwa", [126, 7, 64], BF16, kind="ExternalInput")
    d['stwb'] = nc.dram_tensor("stwb", [42, 7, 64], BF16, kind="ExternalInput")
    d['bn1s'] = nc.dram_tensor("bn1s", [64, 1], F32, kind="ExternalInput")
    d['bn1t'] = nc.dram_tensor("bn1t", [64, 1], F32, kind="ExternalInput")
    d['s1w'] = nc.dram_tensor("s1w", [64, 3, 4, 64], FP8, kind="ExternalInput")
    d['s1wm'] = nc.dram_tensor("s1wm", [128, 3, 4, 64], FP8, kind="ExternalInput")
    d['s1bs'] = nc.dram_tensor("s1bs", [64, 4], F32, kind="ExternalInput")
    d['s1bt'] = nc.dram_tensor("s1bt", [64, 4], F32, kind="ExternalInput")
    for si, (cin, cout, stride) in enumerate(STAGES[1:], start=2):
        kin = (cin + 127) // 128
        kout = (cout + 127) // 128
        kp = min(cin, 128)
        d[f's{si}c1w'] = nc.dram_tensor(f"s{si}c1w", [kp, kin, 9, cout], FP8, kind="ExternalInput")
        d[f's{si}dw'] = nc.dram_tensor(f"s{si}dw", [kp, kin, cout], FP8, kind="ExternalInput")
        d[f's{si}w'] = nc.dram_tensor(f"s{si}w", [128, kout, 9, 3, cout], FP8, kind="ExternalInput")
        d[f's{si}bs'] = nc.dram_tensor(f"s{si}bs", [128, kout, 5], F32, kind="ExternalInput")
        d[f's{si}bt'] = nc.dram_tensor(f"s{si}bt", [128, kout, 5], F32, kind="ExternalInput")
    d['fcw'] = nc.dram_tensor("fcw", [128, 4, 10], F32, kind="ExternalInput")
    d['fcb'] = nc.dram_tensor("fcb", [10, 1], F32, kind="ExternalInput")
    d['fcs'] = nc.dram_tensor("fcs", [10, 1], F32, kind="ExternalInput")
    out_t = nc.dram_tensor("out", [10, B], F32, kind="ExternalOutput")

    with tile.TileContext(nc) as tc:
        import contextlib
        ctx = contextlib.ExitStack()
        wpool = ctx.enter_context(tc.tile_pool(name="w", bufs=1))
        apool = ctx.enter_context(tc.tile_pool(name="a", bufs=1))
        tpool = ctx.enter_context(tc.tile_pool(name="t", bufs=2))
        ppool = ctx.enter_context(tc.tile_pool(name="p", bufs=6, space="PSUM"))
        pfc = ctx.enter_context(tc.tile_pool(name="pfc", bufs=1, space="PSUM"))

        W = {}
        for k in d:
            if k in ('xa0', 'xa08', 'xa016', 'xrh', 'xrl'):
                continue
            W[k] = wpool.tile(list(d[k].shape), d[k].dtype, tag=k, name=k)
            nc.sync.dma_start(W[k][:], d[k][:])

        rpool = ctx.enter_context(tc.tile_pool(name="r", bufs=2))
        res1 = apool.tile([64, SC, 32, 32], F32, tag="res1")
        bin1 = apool.tile([128, CH, 34, 34], FP8, tag="bin1")
        bin1t = apool.tile([128, CH, 34, 34], FP8, tag="bin1t")
        res2 = apool.tile([128, 1, CH, 16, 16], F32, tag="res2")
        bin2 = apool.tile([128, 1, CH, 18, 18], FP8, tag="bin2")
        bin2t = apool.tile([128, 1, CH, 18, 18], FP8, tag="bin2t")
        res3 = apool.tile([128, 2, CH, 8, 8], F32, tag="res3")
        bin3 = apool.tile([128, 2, CH, 10, 10], FP8, tag="bin3")
        bin3t = apool.tile([128, 2, CH, 10, 10], FP8, tag="bin3t")
        res4 = apool.tile([128, 4, CH, 4, 4], F32, tag="res4")
        bin4 = apool.tile([128, 4, CH, 6, 6], FP8, tag="bin4")
        bin4t = apool.tile([128, 4, CH, 6, 6], FP8, tag="bin4t")
        hsg = apool.tile([128, 4, CH, 16], BF16, tag="hsg")
        RkB = apool.tile([126, SC, 32, 38], FP16, tag="RkB")
        nc.vector.memset(RkB[:], 0.0)
        pooled = apool.tile([128, 4, B], F32, tag="pooled")

        for buf in (bin1, bin1t, bin2, bin2t, bin3, bin3t, bin4, bin4t):
            nc.vector.memset(buf[:], 0.0)

        def sgn(out_ap, in_ap, s=1.0, b=0.0):
            nc.scalar.activation(out_ap, in_ap, AT.Sign, bias=b, scale=s)

        def psum(npart, nfree):
            ps = ppool.tile([128, 512], F32, tag="ps")
            return ps[:npart, :nfree]

        for c in range(NCH):
            # ======== STEM + STAGE1 in half-chunks of SC images ========
            for sc in range(2):
                goff = c * CH + sc * SC          # global image offset in B
                loff = sc * SC                   # offset within chunk buffers
                gsl = slice(goff, goff + SC)
                Rm = rpool.tile([126, SC, 32, 38], FP16, tag="Rm")
                Rk = rpool.tile([126, SC, 32, 38], FP16, tag="Rk")
                nc.vector.memset(Rm[:], 0.0)
                nc.vector.memset(Rk[:], 0.0)
                for kh in range(7):
                    y0 = max(0, 3 - kh); y1 = min(31, 34 - kh)
                    sy0 = y0 + kh - 3
                    n_y = y1 - y0 + 1
                    for im in range(SC):
                        gim = goff + im
                        for arr, buf, ro in (('xa0', Rm, 0), ('xrh', Rk, 0),
                                             ('xrl', Rk, 21), ('xa08', Rk, 63),
                                             ('xa016', Rk, 84)):
                            nc.sync.dma_start(
                                buf[ro + 3 * kh:ro + 3 * kh + 3, im,
                                    y0:y1 + 1, 3:35],
                                d[arr][:, gim, sy0:sy0 + n_y, :])
                # duplicate blocks: a0 -> rows 42-62, xrh -> rows 105-125
                nc.sync.dma_start(Rk[42:63], Rm[0:21])
                nc.sync.dma_start(Rk[105:126], Rk[0:21])
                # Rm blocks kw=1..5: a0 shifted left by kw columns (K-merged taps)
                for b in range(1, 6):
                    nc.sync.dma_start(Rm[21 * b:21 * b + 21, :, :, 0:38 - b],
                                      Rm[0:21, :, :, b:38])
                # RkB: [xrh;xrl] shifted by 0/1/2 columns for corrB kw-triples
                nc.sync.dma_start(RkB[0:42], Rk[0:42])
                for b in range(1, 3):
                    nc.sync.dma_start(RkB[42 * b:42 * b + 42, :, :, 0:38 - b],
                                      Rk[0:42, :, :, b:38])
                for img in range(SC):
                    bimg = loff + img            # index into bin1 (chunk-local)
                    for yh in range(2):
                        ys = 16 * yh
                        pv = psum(64, 512).rearrange("p (y x) -> p y x", y=16)
                        nc.tensor.matmul(
                            pv[:], W['stwm6'][:, 0, :],
                            Rm[:, img, ys:ys + 16, 0:32],
                            start=True, stop=False)
                        nc.tensor.matmul(
                            pv[:], W['stwm'][:, 6, :],
                            Rm[0:21, img, ys:ys + 16, 6:38],
                            start=False, stop=False)
                        for kw in range(7):
                            nc.tensor.matmul(
                                pv[:], W['stwa'][:, kw, :],
                                Rk[:, img, ys:ys + 16, kw:kw + 32],
                                start=False, stop=False)
                        for g in range(2):
                            nc.tensor.matmul(
                                pv[:], W['stwb3'][:, g, :],
                                RkB[:, img, ys:ys + 16, 3 * g:3 * g + 32],
                                start=False, stop=False)
                        nc.tensor.matmul(
                            pv[:], W['stwb'][:, 6, :],
                            Rk[0:42, img, ys:ys + 16, 6:38],
                            start=False, stop=True)
                        nc.vector.tensor_scalar(
                            res1[:, img, ys:ys + 16, :], pv[:],
                            W['bn1s'][:, 0:1], W['bn1t'][:, 0:1], OP.mult, OP.add)
                        sgn(bin1[0:64, bimg, 1 + ys:17 + ys, 1:33],
                            res1[:, img, ys:ys + 16, :])
                        sgn(bin1[64:128, bimg, 1 + ys:17 + ys, 0:32],
                            res1[:, img, ys:ys + 16, :])

                # -------- stage 1 (64ch 32x32), two blocks --------
                def conv3_64(pv, src, bimg, yh, widx):
                    for kh in range(3):
                        nc.tensor.matmul(
                            pv[:], W['s1wm'][:, kh, widx, :],
                            src[:, bimg, kh + 16 * yh:kh + 16 * yh + 16, 0:32],
                            start=(kh == 0), stop=False)
                        nc.tensor.matmul(
                            pv[:], W['s1w'][:, kh, widx, :],
                            src[0:64, bimg, kh + 16 * yh:kh + 16 * yh + 16,
                                2:34],
                            start=False, stop=(kh == 2))
                for blk in range(2):
                    c1, c2 = 2 * blk, 2 * blk + 1
                    for img in range(SC):
                        bimg = loff + img
                        for yh in range(2):
                            pv = psum(64, 512).rearrange("p (y x) -> p y x", y=16)
                            conv3_64(pv, bin1, bimg, yh, c1)
                            sgn(bin1t[0:64, bimg, 1 + 16 * yh:17 + 16 * yh, 1:33],
                                pv[:], W['s1bs'][:, c1:c1 + 1],
                                W['s1bt'][:, c1:c1 + 1])
                            sgn(bin1t[64:128, bimg, 1 + 16 * yh:17 + 16 * yh, 0:32],
                                pv[:], W['s1bs'][:, c1:c1 + 1],
                                W['s1bt'][:, c1:c1 + 1])
                    for img in range(SC):
                        bimg = loff + img
                        for yh in range(2):
                            ys = 16 * yh
                            pv = psum(64, 512).rearrange("p (y x) -> p y x", y=16)
                            conv3_64(pv, bin1t, bimg, yh, c2)
                            h2 = tpool.tile([64, 16, 32], F32, tag="h2a")
                            nc.vector.tensor_scalar(
                                h2[:], pv[:], W['s1bs'][:, c2:c2 + 1],
                                W['s1bt'][:, c2:c2 + 1], OP.mult, OP.add)
                            rsl = res1[:, img, ys:ys + 16, :]
                            nc.vector.tensor_tensor(rsl, h2[:], rsl, OP.add)
                            sgn(bin1[0:64, bimg, 1 + ys:17 + ys, 1:33], rsl)
                            sgn(bin1[64:128, bimg, 1 + ys:17 + ys, 0:32], rsl)

            # ======== STAGES 2..4 on the full chunk ========
            def stage(si, cin, cout, get_prev, resp, binc, binct, H):
                kin = (cin + 127) // 128
                kout = (cout + 127) // 128
                px = CH * H * H
                ntile = max(1, px // 512)
                npt = CH // ntile
                Ws, Wt = W[f's{si}bs'], W[f's{si}bt']
                nfree = npt * H * H

                def pview(npart=128):
                    return psum(npart, nfree).rearrange(
                        "p (n y x) -> p n y x", n=npt, y=H)

                def srb(buf, g, ns2, kh, kw):   # stride-1 rhs from own stage buf
                    return buf[:, g, ns2, kh:kh + H, kw:kw + H]

                for tidx in range(ntile):
                    ns2 = slice(tidx * npt, (tidx + 1) * npt)
                    for m in range(kout):
                        mp = slice(128 * m, 128 * m + 128)
                        pv = pview()
                        i = 0
                        for g in range(kin):
                            for kh in range(3):
                                for kw in range(3):
                                    nc.tensor.matmul(
                                        pv[:], W[f's{si}c1w'][:, g, kh * 3 + kw, mp],
                                        get_prev(g, ns2, kh, kw, 2),
                                        start=(i == 0), stop=(i == kin * 9 - 1))
                                    i += 1
                        sgn(binct[:, m, ns2, 1:1 + H, 1:1 + H], pv[:],
                            Ws[:, m, 0:1], Wt[:, m, 0:1])
                        pvd = pview()
                        for g in range(kin):
                            nc.tensor.matmul(
                                pvd[:], W[f's{si}dw'][:, g, mp],
                                get_prev(g, ns2, 1, 1, 2),
                                start=(g == 0), stop=(g == kin - 1))
                        nc.vector.tensor_scalar(
                            resp[:, m, ns2, :, :], pvd[:],
                            Ws[:, m, 1:2], Wt[:, m, 1:2], OP.mult, OP.add)

                for cidx in range(3):
                    srcbuf = binct if cidx in (0, 2) else binc
                    dstbuf = binc if cidx in (0, 2) else binct
                    bnc = 2 + cidx
                    for tidx in range(ntile):
                        ns2 = slice(tidx * npt, (tidx + 1) * npt)
                        for m in range(kout):
                            mp = slice(128 * m, 128 * m + 128)
                            pv = pview()
                            i = 0
                            for g in range(kout):
                                for kh in range(3):
                                    for kw in range(3):
                                        nc.tensor.matmul(
                                            pv[:],
                                            W[f's{si}w'][:, g, kh * 3 + kw, cidx, mp],
                                            srb(srcbuf, g, ns2, kh, kw),
                                            start=(i == 0),
                                            stop=(i == kout * 9 - 1))
                                        i += 1
                            if cidx in (0, 2):
                                h2 = tpool.tile([128, 512], F32, tag="h2b")
                                h2v = h2[:, :nfree].rearrange(
                                    "p (n y x) -> p n y x", n=npt, y=H)
                                nc.vector.tensor_scalar(
                                    h2v[:], pv[:], Ws[:, m, bnc:bnc + 1],
                                    Wt[:, m, bnc:bnc + 1], OP.mult, OP.add)
                                rsl = resp[:, m, ns2, :, :]
                                nc.vector.tensor_tensor(rsl, h2v[:], rsl, OP.add)
                                sgn(dstbuf[:, m, ns2, 1:1 + H, 1:1 + H], rsl)
                            else:
                                sgn(dstbuf[:, m, ns2, 1:1 + H, 1:1 + H], pv[:],
                                    Ws[:, m, bnc:bnc + 1], Wt[:, m, bnc:bnc + 1])

            def prev1(g, ns2, kh, kw, stride):   # from bin1 (4D, chunk-local)
                return bin1[0:64, ns2, kh:kh + 31:2, kw:kw + 31:2]

            def prev2(g, ns2, kh, kw, stride):
                return bin2[:, 0, ns2, kh:kh + 15:2, kw:kw + 15:2]

            def prev3(g, ns2, kh, kw, stride):
                return bin3[:, g, ns2, kh:kh + 7:2, kw:kw + 7:2]

            stage(2, 64, 128, prev1, res2, bin2, bin2t, 16)
            stage(3, 128, 256, prev2, res3, bin3, bin3t, 8)
            stage(4, 256, 512, prev3, res4, bin4, bin4t, 4)

            # ======== POOL ========
            for m in range(4):
                sgn(hsg[:, m, :, :],
                    res4[:, m, :, :, :].rearrange("p n y x -> p n (y x)"))
            acc = tpool.tile([128, 4, CH], F32, tag="poolacc")
            nc.vector.tensor_copy(acc[:], hsg[:, :, :, 0])
            for i in range(1, 16):
                nc.vector.tensor_tensor(acc[:], acc[:], hsg[:, :, :, i], OP.add)
            nc.scalar.mul(pooled[:, :, c * CH:(c + 1) * CH], acc[:], 1.0 / 16.0)

        # ======== FC ========
        psf = pfc.tile([10, B], F32, tag="ps_fc")
        for g in range(4):
            nc.tensor.matmul(psf[:], W['fcw'][:, g, :], pooled[:, g, :],
                             start=(g == 0), stop=(g == 3))
        osb = tpool.tile([10, B], F32, tag="osb")
        nc.vector.tensor_scalar(osb[:], psf[:], W['fcb'][:, 0:1],
                                W['fcs'][:, 0:1], OP.add, OP.mult)
        nc.sync.dma_start(out_t[:], osb[:])
        ctx.close()

    _split_waits(nc)
    return nc


_NC_CACHE = None


def _get_nc():
    global _NC_CACHE
    if _NC_CACHE is None:
        _NC_CACHE = _build()
    return _NC_CACHE


def kernel(x, params, _trace=False):
    rep, shards = _prep_inputs(x, params)
    nc = _get_nc()
    in_maps = []
    for c in range(NCORES):
        m = dict(rep)
        m.update(shards[c])
        in_maps.append(m)
    res = run_bass_kernel_spmd(nc, in_maps, core_ids=list(range(NCORES)))
    outs = [r['out'].T for r in res.results]
    full = np.ascontiguousarray(np.concatenate(outs, axis=0), dtype=np.float32)
    if _trace:
        return full, res
    return full
